# revision 2
# baseline (speedup 1.0000x reference)
# CapsuleNetwork Trainium2 kernel (8-core data parallel, 4 images/core).
#
# Per core, fully software-pipelined over images:
#   conv1 3->256 k9 s1 (im2col K=243, fp8 DoubleRow) -> conv2 256->256 k9 s2
#   (81-tap PSUM accumulation, hybrid bf16/fp8) -> squash -> 3-iter routing.
# conv2 runs as 4 (og, y) phases of one PSUM bank each; taps 14..80 run in
# bf16 (w2 host-scaled x4096, exact pow2), taps 0..13 run in fp8 DoubleRow
# (h1 cast x16 on DVE/gpsimd, w2 x256) -- the tap split keeps the final
# rel-err ~0.015 while shaving ~25% of conv2's PE cycles.  Image b's routing
# instructions are interleaved ("pumped") into image b+1's conv2 tap loop so
# the vector/scalar-bound routing hides under the tensor-bound conv2 stream.
# Startup: w1 + the first im2col ride short pixel-sliced pieces on both HWDGE
# queues so conv1(0) starts ~7us earlier; conv1 walks pixel-chunks n-outer to
# consume them in arrival order.  All squash/routing elementwise math runs on
# DVE/gpsimd (single activation table load); softmax/F' stages are fused into
# whole-tile ops to cut the exposed routing tail of the last image.
import functools
from collections import deque
from contextlib import ExitStack

import numpy as np
import ml_dtypes

import concourse.bass as bass
import concourse.tile as tile
from concourse import bacc
from concourse import mybir
from concourse.bass_utils import run_bass_kernel_spmd

BF = mybir.dt.bfloat16
F32 = mybir.dt.float32
AF = mybir.ActivationFunctionType
AX = mybir.AxisListType

NCORES = 8
B = 4              # images per core
K1 = 243           # 3*9*9 im2col contraction
NPIX1 = 3136       # 56*56 conv1 output pixels
N1CH = 448         # conv1 moving chunk (3136 = 7*448)
PIX = 576          # 24*24 conv2 output pixels
PIX_CHUNKS = [(0, 128), (128, 128), (256, 128), (384, 128), (512, 64)]
R, D, C, O = 32, 8, 10, 16
TF8 = 14           # conv2 taps 0..TF8-1 in fp8 DoubleRow, rest bf16
NTB = 81 - TF8     # bf16 taps


def _build_nc():
    nc = bacc.Bacc("TRN2", target_bir_lowering=False, debug=False)
    F8 = mybir.dt.float8e4
    x_d = nc.declare_dram_parameter("x", [B, 128, 2, NPIX1], F8, isOutput=False)
    w1_d = nc.declare_dram_parameter("w1", [128, 2, 256], F8, isOutput=False)
    b1_d = nc.declare_dram_parameter("b1", [256, 1], F32, isOutput=False)
    w2b_d = nc.declare_dram_parameter("w2b", [2, 2, 128, NTB * 128], BF, isOutput=False)
    w2f_d = nc.declare_dram_parameter("w2f", [2, 128, TF8 * 2 * 128], F8, isOutput=False)
    b2_d = nc.declare_dram_parameter("b2", [256, 1], F32, isOutput=False)
    ws_d = nc.declare_dram_parameter("ws", [256, C * O], BF, isOutput=False)
    wcf_d = nc.declare_dram_parameter("wcf", [80, 2, 256], BF, isOutput=False)
    m80_d = nc.declare_dram_parameter("m80", [80, 2, C], F32, isOutput=False)
    m80b_d = nc.declare_dram_parameter("m80b", [80, 2, C], BF, isOutput=False)
    m580_d = nc.declare_dram_parameter("m580", [5, 80], F32, isOutput=False)
    maskg_d = nc.declare_dram_parameter("maskg", [2, 128, R * C], F32, isOutput=False)
    idf_d = nc.declare_dram_parameter("idf", [128, 128], F32, isOutput=False)
    idb_d = nc.declare_dram_parameter("idb", [128, 128], BF, isOutput=False)
    vout_d = nc.declare_dram_parameter("v_out", [B * C, O], F32, isOutput=True)

    with tile.TileContext(nc) as tc, ExitStack() as ctx:
        consts = ctx.enter_context(tc.tile_pool(name="consts", bufs=1))
        w1t = consts.tile([128, 2, 256], mybir.dt.float8e4, tag="w1t",
                          name="w1t")
        b1t = [consts.tile([128, 1], F32, tag=f"b1_{m}", name=f"b1_{m}") for m in range(2)]
        b2t = [consts.tile([128, 1], F32, tag=f"b2_{m}", name=f"b2_{m}") for m in range(2)]
        ws_t = [consts.tile([128, C * O], BF, tag=f"ws{m}", name=f"ws{m}") for m in range(2)]
        wcf = consts.tile([80, 2, 256], BF, tag="wcf", name="wcf")
        m80 = consts.tile([80, 2, C], F32, tag="m80", name="m80")
        m80b = consts.tile([80, 2, C], BF, tag="m80b", name="m80b")
        m580 = consts.tile([5, 80], F32, tag="m580", name="m580")
        idf = consts.tile([128, 128], F32, tag="idf", name="idf")
        idb = consts.tile([128, 128], BF, tag="idb", name="idb")
        maskg = [consts.tile([128, R * C], F32, tag=f"mg{m}", name=f"mg{m}")
                 for m in range(2)]
        # fast-inverse-sqrt magic seed (0x5f3759df) as an f32-bit pattern
        magic = consts.tile([128, 32], F32, tag="magic", name="magic")
        nc.vector.memset(
            magic, float(np.uint32(0x5F3759DF).view(np.float32)))
        magic5 = consts.tile([128, 5, 32], F32, tag="magic5", name="magic5")
        nc.vector.memset(
            magic5, float(np.uint32(0x5F3759DF).view(np.float32)))

        def dve_rsqrt(y, x, p, n, tmp_tag, iters=2):
            """y[:p,:n] = 1/sqrt(x[:p,:n]) on DVE only (bit trick +
            Newton).  No scalar engine -> no act-table thrash."""
            t = dpool.tile([128, 32], F32, tag=f"{tmp_tag}t", name=f"{tmp_tag}t")
            nc.vector.tensor_scalar(
                y.bitcast(mybir.dt.uint32),
                x.bitcast(mybir.dt.uint32), 1, None,
                op0=mybir.AluOpType.logical_shift_right)
            nc.vector.tensor_tensor(
                y.bitcast(mybir.dt.uint32),
                magic[:p, :n].bitcast(mybir.dt.uint32),
                y.bitcast(mybir.dt.uint32),
                op=mybir.AluOpType.subtract)
            for _ in range(iters):  # y *= 1.5 - 0.5*x*y*y
                nc.vector.tensor_mul(t[:p, :n], y, y)
                nc.vector.tensor_mul(t[:p, :n], t[:p, :n], x)
                nc.vector.tensor_scalar(
                    t[:p, :n], t[:p, :n], -0.5, 1.5,
                    op0=mybir.AluOpType.mult, op1=mybir.AluOpType.add)
                nc.vector.tensor_mul(y, y, t[:p, :n])

        # ---- persistent pools (whole-kernel lifetime, ring-buffered) ----
        h1pool = ctx.enter_context(tc.tile_pool(name="h1p", bufs=2))
        h8pool = ctx.enter_context(tc.tile_pool(name="h8p", bufs=2))
        impool = ctx.enter_context(tc.tile_pool(name="imp", bufs=2))
        w2pool = ctx.enter_context(tc.tile_pool(name="w2p", bufs=1))
        crawpool = ctx.enter_context(tc.tile_pool(name="crawp", bufs=2))
        capspool = ctx.enter_context(tc.tile_pool(name="capsp", bufs=2))
        rpool = ctx.enter_context(tc.tile_pool(name="rpool", bufs=2))
        dpool = ctx.enter_context(tc.tile_pool(name="dtmp", bufs=4))
        pmpool = ctx.enter_context(tc.tile_pool(name="pmp", bufs=5))
        cpsum = ctx.enter_context(tc.tile_pool(name="cpsum", bufs=4, space="PSUM"))
        tps = ctx.enter_context(tc.tile_pool(name="tps", bufs=2, space="PSUM"))
        rps = ctx.enter_context(tc.tile_pool(name="rps", bufs=2, space="PSUM"))

        w2bt = [[w2pool.tile([128, NTB, 128], BF, tag=f"w2b_{ig}_{og}",
                             name=f"w2b_{ig}_{og}")
                 for og in range(2)] for ig in range(2)]
        w2ft = [w2pool.tile([128, TF8, 2, 128], mybir.dt.float8e4,
                            tag=f"w2f_{og}", name=f"w2f_{og}")
                for og in range(2)]

        # ================= DMA issue block =================
        # Two HWDGE queues (sync + scalar); scalar's queue stays SHORT (5
        # early issues, no WAR waits) so its relu/exp compute never queues
        # behind DMA issues.  w1 + pixel-sliced im0 pieces lead on both
        # queues so conv1(0) can start ~11us in; w2 og0 pieces follow in
        # tap-consumption order (slice-precise dep tracking unblocks
        # conv2's taps as pieces land).  gpsimd/SWDGE takes the small
        # routing consts.
        for m in range(2):
            nc.gpsimd.dma_start(b1t[m], b1_d[m * 128:(m + 1) * 128, :])
        for m in range(2):
            nc.gpsimd.dma_start(b2t[m], b2_d[m * 128:(m + 1) * 128, :])
        nc.gpsimd.dma_start(idf, idf_d[:, :])
        nc.gpsimd.dma_start(idb, idb_d[:, :])
        for m in range(2):
            nc.gpsimd.dma_start(maskg[m], maskg_d[m])
        for m in range(2):
            nc.gpsimd.dma_start(ws_t[m], ws_d[m * 128:(m + 1) * 128, :])
        nc.gpsimd.dma_start(wcf, wcf_d[:, :, :])
        nc.gpsimd.dma_start(m80, m80_d[:, :, :])
        nc.gpsimd.dma_start(m80b, m80b_d[:, :, :])
        nc.gpsimd.dma_start(m580, m580_d[:, :])

        im = [None] * B

        def issue_im(b):
            imt = impool.tile([128, 2, NPIX1], mybir.dt.float8e4, tag="imA",
                              name="imA")
            nc.sync.dma_start(imt.rearrange("p j n -> p (j n)"),
                              x_d[b].rearrange("p j n -> p (j n)"))
            im[b] = imt

        def w2b_piece(eng, ig, og, t0, t1):
            a, bb = t0 - TF8, t1 - TF8
            eng.dma_start(
                w2bt[ig][og][:, a:bb].rearrange("p t m -> p (t m)"),
                w2b_d[ig, og, :, a * 128:bb * 128])

        PXS = 1568  # im0 pixel split point (3.5 of 7 conv1 chunks)
        im0 = impool.tile([128, 2, NPIX1], mybir.dt.float8e4, tag="imA",
                          name="imA")
        im[0] = im0
        # scalar queue: w1 -> im0 front half -> first og0 bf16 pieces
        nc.scalar.dma_start(w1t.rearrange("p j n -> p (j n)"),
                            w1_d.rearrange("p j n -> p (j n)"))
        nc.scalar.dma_start(im0[0:64, :, 0:PXS], x_d[0, 0:64, :, 0:PXS])
        nc.scalar.dma_start(im0[64:128, :, 0:PXS], x_d[0, 64:128, :, 0:PXS])
        w2b_piece(nc.scalar, 0, 0, TF8, 46)
        w2b_piece(nc.scalar, 1, 0, 46, 81)
        # sync queue: im0 back half -> rest of og0 -> fp8 -> og1 -> im1-3
        nc.sync.dma_start(im0[0:64, :, PXS:NPIX1], x_d[0, 0:64, :, PXS:NPIX1])
        nc.sync.dma_start(im0[64:128, :, PXS:NPIX1],
                          x_d[0, 64:128, :, PXS:NPIX1])
        w2b_piece(nc.sync, 1, 0, TF8, 46)
        w2b_piece(nc.sync, 0, 0, 46, 81)
        nc.sync.dma_start(w2ft[0].rearrange("p t j m -> p (t j m)"), w2f_d[0])
        issue_im(1)
        nc.sync.dma_start(w2ft[1].rearrange("p t j m -> p (t j m)"), w2f_d[1])
        w2b_piece(nc.sync, 0, 1, TF8, 46)
        w2b_piece(nc.sync, 1, 1, TF8, 46)
        w2b_piece(nc.sync, 0, 1, 46, 81)
        w2b_piece(nc.sync, 1, 1, 46, 81)
        issue_im(2)
        issue_im(3)

        # ================= stage pump =================
        pending = deque()
        tapctr = [0]
        STAGE_START, STAGE_EVERY = 16, 72

        def pump():
            tapctr[0] += 1
            if (pending and tapctr[0] >= STAGE_START
                    and (tapctr[0] - STAGE_START) % STAGE_EVERY == 0
                    and tapctr[0] >= pending[0][1]):
                pending.popleft()[0]()

        def drain():
            while pending:
                pending.popleft()[0]()

        # ================= per-image phases =================
        def conv1(b):
            # fp8 DoubleRow: K=256 (two 128-row k-groups) per instruction;
            # host pre-scales x by 16 and w1 by 256, undone by the relu
            # drain's 2^-12 activation scale.  n-outer so pixel chunks are
            # consumed in DMA arrival order.
            imt = im[b]
            h1t = h1pool.tile([128, 2, 56, 2, 28], BF, tag="h1t", name="h1t")
            for n in range(7):
                for m in range(2):
                    ps = cpsum.tile([128, 8, 56], F32, tag="cps", name="c1ps")
                    nc.tensor.matmul(ps, w1t[:, :, m * 128:(m + 1) * 128],
                                     imt[:, :, n * N1CH:(n + 1) * N1CH],
                                     start=True, stop=True,
                                     perf_mode=mybir.MatmulPerfMode.DoubleRow)
                    # single drain per (n, m): phase-interleave via strided
                    # view; alternate ACT/DVE so drains never pace conv1.
                    ps_v = ps.rearrange("p r (x2 ph) -> p r ph x2", ph=2)
                    dst = h1t[:, m, 8 * n:8 * n + 8, :, :]
                    if n % 2 == 0:
                        nc.scalar.activation(dst, ps_v, AF.Relu,
                                             bias=b1t[m], scale=2.0 ** -12)
                    else:
                        tmp = dpool.tile([128, 8, 2, 28], F32, tag="c1t",
                                         name="c1t")
                        nc.vector.tensor_scalar(
                            tmp, ps_v, 2.0 ** -12, b1t[m],
                            op0=mybir.AluOpType.mult,
                            op1=mybir.AluOpType.add)
                        nc.vector.tensor_scalar_max(dst, tmp, 0.0)
            return h1t

        def h1cast(h1t):
            # fp8 copy of h1 (x16) for the DoubleRow taps; runs on DVE +
            # gpsimd under conv2's leading bf16 taps.
            hf = h8pool.tile([128, 2, 56, 2, 28], mybir.dt.float8e4,
                             tag="h1f8", name="h1f8")
            nc.vector.tensor_scalar_mul(hf[:, 0], h1t[:, 0], 16.0)
            nc.gpsimd.tensor_scalar_mul(hf[:, 1], h1t[:, 1], 16.0)
            return hf

        def conv2(b, h1t, h1f8, craw):
            # 4 single-bank phases (og, y).  bf16 taps first (w2 x4096),
            # fp8 DoubleRow taps last (so the h1 cast hides under bf16);
            # both accumulate at the same 2^12 scale, undone in the drain.
            tapctr[0] = 0
            for og in range(2):
                for y in range(2):
                    ps = cpsum.tile([128, 288], F32, tag="cps", name="c2ps")
                    for t81 in range(TF8, 81):
                        kh, kw = t81 // 9, t81 % 9
                        for ig in range(2):
                            rhs = h1t[:, ig,
                                      kh + 24 * y:kh + 24 * y + 24:2,
                                      kw % 2, kw // 2:kw // 2 + 24]
                            nc.tensor.matmul(
                                ps, w2bt[ig][og][:, t81 - TF8, :], rhs,
                                start=(t81 == TF8 and ig == 0), stop=False)
                            pump()
                    for t81 in range(TF8):
                        kh, kw = t81 // 9, t81 % 9
                        rhs = h1f8[:, :,
                                   kh + 24 * y:kh + 24 * y + 24:2,
                                   kw % 2, kw // 2:kw // 2 + 24]
                        nc.tensor.matmul(
                            ps, w2ft[og][:, t81], rhs,
                            start=False, stop=(t81 == TF8 - 1),
                            perf_mode=mybir.MatmulPerfMode.DoubleRow)
                        pump()
                    # drain on DVE (keeps scalar's act table on Exp)
                    nc.vector.tensor_scalar(
                        craw[:, og, y * 288:(y + 1) * 288], ps,
                        2.0 ** -12, b2t[og],
                        op0=mybir.AluOpType.mult, op1=mybir.AluOpType.add)

        sqst = {}

        def squash_og0(b, craw):
            # og0 half of the squash front-end, pumped into conv2-b's og1
            # tap window (min_tap guards the og0 drain dependency).
            def f():
                pms = [pmpool.tile([128, 256], F32, tag="pm", name="pm")
                       for _ in PIX_CHUNKS]
                nsqs = dpool.tile([128, 5, R], F32, tag="nsqs", name="nsqs")
                for k, (p0, ln) in enumerate(PIX_CHUNKS):
                    tp = tps.tile([128, 128], F32, tag="tp", name="tp")
                    nc.tensor.transpose(tp[:ln, :], craw[:, 0, p0:p0 + ln],
                                        idf)
                    nc.scalar.activation(pms[k][:ln, 0:128], tp[:ln, :],
                                         AF.Copy)
                for k, (p0, ln) in enumerate(PIX_CHUNKS):
                    pm3 = pms[k].rearrange("p (r i) -> p r i", i=D)
                    sq = dpool.tile([128, 16, D], F32, tag="sqh", name="sqh")
                    eng = nc.gpsimd if k < 2 else nc.vector
                    eng.tensor_mul(sq[:ln], pm3[:ln, 0:16], pm3[:ln, 0:16])
                    nc.vector.reduce_sum(nsqs[:ln, k, 0:16], sq[:ln],
                                         axis=AX.X)
                sqst[b] = (pms, nsqs)
            return f

        def squash_finish(b, craw):
            # og1 half + packed scale chain + backward transposes.
            pms, nsqs = sqst.pop(b)
            capsbf = capspool.tile([128, 5, 256], BF, tag="cbf", name="cbf")
            capsT = capspool.tile([128, 2, PIX], BF, tag="cT", name="cT")
            capsum = capspool.tile([128, 2], F32, tag="csum", name="csum")
            for k, (p0, ln) in enumerate(PIX_CHUNKS):
                tp = tps.tile([128, 128], F32, tag="tp", name="tp")
                nc.tensor.transpose(tp[:ln, :], craw[:, 1, p0:p0 + ln], idf)
                nc.vector.tensor_copy(pms[k][:ln, 128:256], tp[:ln, :])
            for k, (p0, ln) in enumerate(PIX_CHUNKS):
                pm3 = pms[k].rearrange("p (r i) -> p r i", i=D)
                sq = dpool.tile([128, 16, D], F32, tag="sqh", name="sqh")
                eng = nc.gpsimd if k < 2 else nc.vector
                eng.tensor_mul(sq[:ln], pm3[:ln, 16:32], pm3[:ln, 16:32])
                nc.vector.reduce_sum(nsqs[:ln, k, 16:32], sq[:ln],
                                     axis=AX.X)
            # scale = n * rsqrt(n) / (1+n), all on DVE, chunk-packed
            sqas = dpool.tile([128, 5, R], F32, tag="sqas", name="sqas")
            rys = dpool.tile([128, 5, R], F32, tag="rys", name="rys")
            nc.vector.tensor_scalar(
                rys.bitcast(mybir.dt.uint32), nsqs.bitcast(mybir.dt.uint32),
                1, None, op0=mybir.AluOpType.logical_shift_right)
            nc.vector.tensor_tensor(
                rys.bitcast(mybir.dt.uint32),
                magic5.bitcast(mybir.dt.uint32),
                rys.bitcast(mybir.dt.uint32), op=mybir.AluOpType.subtract)
            nt = dpool.tile([128, 5, R], F32, tag="nt", name="nt")
            nc.vector.tensor_mul(nt, rys, rys)
            nc.vector.tensor_mul(nt, nt, nsqs)
            nc.vector.tensor_scalar(
                nt, nt, -0.5, 1.5,
                op0=mybir.AluOpType.mult, op1=mybir.AluOpType.add)
            nc.vector.tensor_mul(rys, rys, nt)
            nc.vector.tensor_scalar_add(sqas, nsqs, 1.0)
            nc.vector.reciprocal(sqas, sqas)
            nc.vector.tensor_mul(sqas, sqas, rys)
            nc.vector.tensor_mul(sqas, nsqs, sqas)
            for k, (p0, ln) in enumerate(PIX_CHUNKS):
                pm3 = pms[k].rearrange("p (r i) -> p r i", i=D)
                cbf3 = capsbf[:, k].rearrange("p (r i) -> p r i", i=D)
                eng = nc.gpsimd if k < 2 else nc.vector
                eng.tensor_mul(
                    cbf3[:ln], pm3[:ln],
                    sqas[:ln, k].unsqueeze(2).broadcast_to([ln, R, D]))
            for k, (p0, ln) in enumerate(PIX_CHUNKS):
                for og in range(2):
                    tb = tps.tile([128, 128], BF, tag="tp", name="tb")
                    nc.tensor.transpose(
                        tb[:, :ln],
                        capsbf[:ln, k, og * 128:(og + 1) * 128],
                        idb[:ln, :ln])
                    if og == 0:
                        nc.scalar.activation(capsT[:, og, p0:p0 + ln],
                                             tb[:, :ln], AF.Copy)
                    else:
                        nc.vector.tensor_copy(capsT[:, og, p0:p0 + ln],
                                              tb[:, :ln])
            for g in range(2):
                nc.vector.reduce_sum(capsum[:, g:g + 1], capsT[:, g],
                                     axis=AX.X)
            return capsbf, capsT, capsum

        # ================= routing (per image, staged) =================
        def register_routing(b, capsbf, capsT, capsum):
            st = {}
            blog = rpool.tile([128, 5, R, C], F32, tag="blog", name="blog")
            # chunk 4 covers only 64 partitions; zero the rest once so the
            # fused whole-tile exp/reduce stay finite there.
            nc.gpsimd.memset(blog[64:128, 4], 0.0)

            def sv_t4(it, last=False):
                # (c,o)-packed layout: partition p = c_local*16+o, halves
                # h=0 (c 0-4) / h=1 (c 5-9).  The per-class s/T4 matmuls
                # batch into 4 + 4 wide ones; the c==c' diagonal is pulled
                # out with a mask multiply + reduce on DVE.
                def f():
                    sF = dpool.tile([80, 2], F32, tag="sF", name="sF")
                    if it == 0:
                        csb = dpool.tile([128, 2], BF, tag="csb", name="csb")
                        nc.vector.tensor_scalar_mul(csb, capsum, 1.0 / C)
                        sps = rps.tile([80, 2], F32, tag="rps", name="sps0")
                        for h in range(2):
                            for m in range(2):
                                nc.tensor.matmul(
                                    sps[:, h:h + 1],
                                    ws_t[m][:, 80 * h:80 * (h + 1)],
                                    csb[:, m:m + 1],
                                    start=(m == 0), stop=(m == 1))
                        nc.vector.tensor_copy(sF, sps)
                    else:
                        Gp = st['Gp']
                        for h in range(2):
                            sps = rps.tile([80, C], F32, tag="rps",
                                           name=f"sps{h}")
                            for m in range(2):
                                nc.tensor.matmul(
                                    sps, ws_t[m][:, 80 * h:80 * (h + 1)],
                                    Gp[m], start=(m == 0), stop=(m == 1))
                            fm = dpool.tile([80, C], F32, tag="sfm",
                                            name="sfm")
                            nc.vector.tensor_mul(fm, sps, m80[:, h])
                            nc.vector.reduce_sum(sF[:, h:h + 1], fm,
                                                 axis=AX.X)
                    # squash on the packed layout: per-class norms via a
                    # block-ones matmul, scale chain on [5, 2], broadcast
                    # back via a K=5 matmul.  All elementwise on DVE.
                    sq2 = dpool.tile([80, 2], F32, tag="sq2", name="sq2")
                    nc.vector.tensor_mul(sq2, sF, sF)
                    n2ps = rps.tile([5, 2], F32, tag="rps", name="n2ps")
                    for h in range(2):
                        nc.tensor.matmul(n2ps[:, h:h + 1], m80[:, 0, :5],
                                         sq2[:, h:h + 1],
                                         start=True, stop=True)
                    n2 = dpool.tile([5, 2], F32, tag="n2", name="n2")
                    nc.vector.tensor_copy(n2, n2ps)
                    ry = dpool.tile([128, 16], F32, tag="ry", name="ry")
                    dve_rsqrt(ry[:5, :2], n2, 5, 2, "vr", iters=1)
                    a2 = dpool.tile([5, 2], F32, tag="a2", name="a2")
                    nc.vector.tensor_scalar_add(a2, n2, 1.0)
                    nc.vector.reciprocal(a2, a2)
                    nc.vector.tensor_mul(a2, a2, ry[:5, :2])
                    nc.vector.tensor_mul(a2, n2, a2)
                    scps = rps.tile([80, 2], F32, tag="rps", name="scps")
                    for h in range(2):
                        nc.tensor.matmul(scps[:, h:h + 1], m580,
                                         a2[:, h:h + 1],
                                         start=True, stop=True)
                    if last:
                        vff = dpool.tile([80, 2], F32, tag="vff", name="vff")
                        nc.vector.tensor_mul(vff, sF, scps)
                        vT = rps.tile([2, 80], F32, tag="rps", name="vT")
                        nc.tensor.transpose(vT, vff, idf[:80, :80])
                        vout = rpool.tile([2, 80], F32, tag="vout",
                                          name="vout")
                        nc.vector.tensor_copy(vout, vT)
                        nc.sync.dma_start(
                            vout_d[b * C:(b + 1) * C, :]
                            .rearrange("(h f) o -> h (f o)", f=5), vout)
                        return
                    vF2 = rpool.tile([80, 2], BF, tag="vF2", name="vF2")
                    nc.vector.tensor_mul(vF2, sF, scps)
                    # T4: rhs = vF broadcast masked to [80, C] per half,
                    # contraction over the packed (c,o) dim in 2 halves.
                    T4 = [rpool.tile([128, R * C], BF, tag=f"T4_{m}",
                                     name=f"T4_{m}") for m in range(2)]
                    vm = dpool.tile([80, 2, C], BF, tag="vm", name="vm")
                    nc.vector.tensor_mul(
                        vm, vF2.unsqueeze(2).broadcast_to([80, 2, C]), m80b)
                    for m in range(2):
                        t4 = rps.tile([128, C], F32, tag="rps", name="t4")
                        for h in range(2):
                            nc.tensor.matmul(
                                t4, wcf[:, h, m * 128:(m + 1) * 128],
                                vm[:, h], start=(h == 0), stop=(h == 1))
                        data = t4.unsqueeze(1).broadcast_to([128, R, C])
                        mk = maskg[m].rearrange("p (r c) -> p r c", c=C)
                        nc.vector.tensor_mul(
                            T4[m].rearrange("p (r c) -> p r c", c=C),
                            data, mk)
                    st['T4'] = T4
                return f

            def dlstage(it):
                def f():
                    T4 = st['T4']
                    for k, (p0, ln) in enumerate(PIX_CHUNKS):
                        dl = rps.tile([128, R, C], F32, tag="rps", name="dl")
                        for kc in range(2):
                            nc.tensor.matmul(
                                dl[:ln], capsT[:, kc, p0:p0 + ln],
                                T4[kc], start=(kc == 0), stop=(kc == 1))
                        if it == 0:
                            nc.vector.tensor_copy(blog[:ln, k], dl[:ln])
                        else:
                            nc.vector.tensor_add(blog[:ln, k], blog[:ln, k],
                                                 dl[:ln])
                return f

            def efstage():
                # fused softmax-weight + F' stage: whole-tile exp/reduce/
                # reciprocal, 2-way split weighting, then the 5x2 F4
                # accumulation matmuls.
                e = rpool.tile([128, 5, R, C], BF, tag="e", name="e")
                cwt = rpool.tile([128, 5, R, D], BF, tag="cw", name="cw")
                F4 = [rps.tile([128, R * C], F32, tag="rps", name=f"F4_{m}")
                      for m in range(2)]
                dens = dpool.tile([128, 5, R], F32, tag="dens", name="dens")
                nc.scalar.activation(e, blog, AF.Exp)
                nc.vector.reduce_sum(dens, e, axis=AX.X)
                nc.vector.reciprocal(dens, dens)
                cbf4 = capsbf.rearrange("p k (r i) -> p k r i", i=D)
                nc.gpsimd.tensor_mul(
                    cwt[:, 0:2], cbf4[:, 0:2],
                    dens[:, 0:2].unsqueeze(3).broadcast_to([128, 2, R, D]))
                nc.vector.tensor_mul(
                    cwt[:, 2:5], cbf4[:, 2:5],
                    dens[:, 2:5].unsqueeze(3).broadcast_to([128, 3, R, D]))
                for k, (p0, ln) in enumerate(PIX_CHUNKS):
                    cwf = cwt[:, k].rearrange("p r i -> p (r i)")
                    ef = e[:, k].rearrange("p r c -> p (r c)")
                    for m in range(2):
                        nc.tensor.matmul(F4[m],
                                         cwf[:ln, m * 128:(m + 1) * 128],
                                         ef[:ln],
                                         start=(k == 0), stop=(k == 4))
                Gp = [rpool.tile([128, C], BF, tag=f"G{m}", name=f"G{m}")
                      for m in range(2)]
                for m in range(2):
                    fm = dpool.tile([128, R * C], BF, tag="fm", name="fm")
                    nc.vector.tensor_mul(fm, F4[m], maskg[m])
                    gf = dpool.tile([128, C], F32, tag="gf", name="gf")
                    nc.vector.reduce_sum(
                        gf, fm.rearrange("p (r c) -> p c r", c=C), axis=AX.X)
                    nc.vector.tensor_copy(Gp[m], gf)
                st['Gp'] = Gp

            pending.extend([
                (sv_t4(0), 0), (dlstage(0), 0), (efstage, 0),
                (sv_t4(1), 0), (dlstage(1), 0), (efstage, 0),
                (sv_t4(2, last=True), 0),
            ])

        # ================= main pipeline =================
        h1s = {0: conv1(0)}
        for b in range(B):
            craw = crawpool.tile([128, 2, PIX], F32, tag="craw", name="craw")
            hf = h1cast(h1s[b])
            pending.append((squash_og0(b, craw), 300))
            conv2(b, h1s[b], hf, craw)
            if b + 1 < B:
                h1s[b + 1] = conv1(b + 1)
            drain()  # safety: all pumped stages must be fully emitted
            cbs = squash_finish(b, craw)
            register_routing(b, *cbs)
        drain()  # routing of the last image (exposed tail)

    nc.compile()
    return nc


@functools.lru_cache(maxsize=1)
def _get_nc():
    return _build_nc()


def _prep_consts(conv1_w, conv1_b, conv2_w, conv2_b, route_w):
    bf = ml_dtypes.bfloat16
    f8 = ml_dtypes.float8_e4m3
    f32 = np.float32
    w1 = np.zeros((256, 256), f32)
    w1[:K1] = conv1_w.astype(f32).transpose(1, 2, 3, 0).reshape(K1, 256)
    # DoubleRow lhsT layout [p, j, m] = w1[j*128+p, m], fp8 with x256 scale
    w1dr = np.clip(w1 * 256.0, -240, 240).reshape(2, 128, 256).transpose(1, 0, 2)
    w2 = conv2_w.astype(f32).reshape(2, 128, 2, 128, 81)  # [og, mo, ig, ki, tap]
    # bf16 taps TF8..80: [ig, og, ki, tap, mo], x4096 (exact pow2) so they
    # accumulate at the same scale as the fp8 (x16 * x256) taps
    w2b = (w2[..., TF8:].transpose(2, 0, 3, 4, 1) * 4096.0)
    # fp8 taps 0..TF8-1: [og, ki, tap, ig, mo], x256
    w2f = np.clip(w2[..., :TF8].transpose(0, 3, 4, 2, 1) * 256.0, -240, 240)
    ws = route_w.astype(f32).transpose(0, 2, 1, 3).reshape(256, C * O)
    # wcf[(c_l,o), h, q] = wcob[o, 5h+c_l, q]; wcob[o,c,q] = route_w view
    wcob = route_w.astype(f32).transpose(3, 1, 0, 2).reshape(O, C, 256)
    wcf = np.zeros((80, 2, 256), f32)
    for cl in range(5):
        for o in range(O):
            for h in range(2):
                wcf[cl * 16 + o, h] = wcob[o, 5 * h + cl]
    # m80[p=(c_l,o), h, c'] = (c' == 5h + c_l)
    m80 = np.zeros((80, 2, C), f32)
    for cl in range(5):
        for h in range(2):
            m80[cl * 16:(cl + 1) * 16, h, 5 * h + cl] = 1.0
    # m580[j, p] = (j == p//16)
    m580 = np.zeros((5, 80), f32)
    for j in range(5):
        m580[j, j * 16:(j + 1) * 16] = 1.0
    maskg = np.zeros((2, 128, R * C), f32)
    for m in range(2):
        for j in range(128):
            r = m * 16 + j // D
            maskg[m, j, r * C:(r + 1) * C] = 1.0
    return {
        "w1": np.ascontiguousarray(w1dr).astype(f8),
        "b1": np.ascontiguousarray(conv1_b.astype(f32).reshape(256, 1)),
        "w2b": np.ascontiguousarray(w2b).reshape(2, 2, 128, NTB * 128).astype(bf),
        "w2f": np.ascontiguousarray(w2f).reshape(2, 128, TF8 * 2 * 128).astype(f8),
        "b2": np.ascontiguousarray(conv2_b.astype(f32).reshape(256, 1)),
        "ws": np.ascontiguousarray(ws).astype(bf),
        "wcf": np.ascontiguousarray(wcf).astype(bf),
        "m80": m80,
        "m80b": np.ascontiguousarray(m80).astype(bf),
        "m580": m580,
        "idf": np.eye(128, dtype=f32),
        "idb": np.eye(128, dtype=f32).astype(bf),
        "maskg": maskg,
    }


def _ensure_ntff_hook():
    """The agent image's antenv lacks axon_hooks; shim it so trace=True works."""
    import sys
    import types
    try:
        from antenv import axon_hooks  # noqa: F401
        return
    except ImportError:
        pass
    mod = types.ModuleType("antenv.axon_hooks")
    _h = [None]
    mod.get_axon_ntff_profile_hook = lambda: _h[0]
    mod.set_axon_ntff_profile_hook = lambda h: _h.__setitem__(0, h)
    sys.modules["antenv.axon_hooks"] = mod
    try:
        from trn_agent_boot.trn_boot import _ntff_profile_via_ctypes
        mod.set_axon_ntff_profile_hook(
            _ntff_profile_via_ctypes("/opt/axon/libaxon_pjrt.so"))
    except Exception as e:  # degrade: trace skipped, run still works
        print(f"ntff hook shim failed: {e}")


def run(x, conv1_w, conv1_b, conv2_w, conv2_b, route_w, trace=False, cores=NCORES):
    if trace:
        _ensure_ntff_hook()
    x = np.asarray(x, np.float32)
    nb = x.shape[0]
    consts = _prep_consts(np.asarray(conv1_w), np.asarray(conv1_b),
                          np.asarray(conv2_w), np.asarray(conv2_b),
                          np.asarray(route_w))
    win = np.lib.stride_tricks.sliding_window_view(x, (9, 9), axis=(2, 3))
    xb = (win.transpose(0, 1, 4, 5, 2, 3)          # [b, c, kh, kw, y, x]
          .reshape(nb, K1, NPIX1))
    # fp8 DoubleRow layout [b, p, j, n], rows 243..255 zero, x16 scale
    xq = np.zeros((nb, 256, NPIX1), np.float32)
    xq[:, :K1] = np.clip(xb * 16.0, -240, 240)
    xq = np.ascontiguousarray(
        xq.reshape(nb, 2, 128, NPIX1).transpose(0, 2, 1, 3)
    ).astype(ml_dtypes.float8_e4m3)
    assert nb == B * cores
    in_maps = []
    for cid in range(cores):
        m = dict(consts)
        m["x"] = np.ascontiguousarray(xq[cid * B:(cid + 1) * B])
        in_maps.append(m)
    res = run_bass_kernel_spmd(_get_nc(), in_maps, list(range(cores)), trace=trace)
    out = np.concatenate([r["v_out"].reshape(B, C, O) for r in res.results], axis=0)
    return out.astype(np.float32), res


def kernel(x, conv1_w, conv1_b, conv2_w, conv2_b, route_w):
    out, _ = run(x, conv1_w, conv1_b, conv2_w, conv2_b, route_w, trace=False)
    return out


# revision 4
# speedup vs baseline: 1.3647x; 1.3647x over previous
# CapsuleNetwork Trainium2 kernel (8-core data parallel, 4 images/core).
#
# Per core, fully software-pipelined over images:
#   conv1 3->256 k9 s1 (im2col K=243, fp8 DoubleRow) -> conv2 256->256 k9 s2
#   (81-tap PSUM accumulation, hybrid bf16/fp8) -> squash -> 3-iter routing.
# conv2 runs as 4 (og, y) phases of one PSUM bank each; taps 14..80 run in
# bf16 (w2 host-scaled x4096, exact pow2), taps 0..13 run in fp8 DoubleRow
# (h1 cast x16 on DVE/gpsimd, w2 x256) -- the tap split keeps the final
# rel-err ~0.015 while shaving ~25% of conv2's PE cycles.  Image b's routing
# instructions are interleaved ("pumped") into image b+1's conv2 tap loop so
# the vector/scalar-bound routing hides under the tensor-bound conv2 stream.
# Startup: w1 + the first im2col ride short pixel-sliced pieces on both HWDGE
# queues so conv1(0) starts ~7us earlier; conv1 walks pixel-chunks n-outer to
# consume them in arrival order.  All squash/routing elementwise math runs on
# DVE/gpsimd (single activation table load); softmax/F' stages are fused into
# whole-tile ops to cut the exposed routing tail of the last image.
import functools
from collections import deque
from contextlib import ExitStack

import numpy as np
import ml_dtypes

import concourse.bass as bass
import concourse.tile as tile
from concourse import bacc
from concourse import mybir
from concourse.bass_utils import run_bass_kernel_spmd

BF = mybir.dt.bfloat16
F32 = mybir.dt.float32
AF = mybir.ActivationFunctionType
AX = mybir.AxisListType

NCORES = 8
B = 4              # images per core
K1 = 243           # 3*9*9 im2col contraction
NPIX1 = 3136       # 56*56 conv1 output pixels
N1CH = 448         # conv1 moving chunk (3136 = 7*448)
PIX = 576          # 24*24 conv2 output pixels
PIX_CHUNKS = [(0, 128), (128, 128), (256, 128), (384, 128), (512, 64)]
R, D, C, O = 32, 8, 10, 16
TF8 = 14           # conv2 taps 0..TF8-1 in fp8 DoubleRow, rest bf16
NTB = 81 - TF8     # bf16 taps


def _build_nc():
    nc = bacc.Bacc("TRN2", target_bir_lowering=False, debug=False)
    F8 = mybir.dt.float8e4
    x_d = nc.declare_dram_parameter("x", [B, 128, 2, NPIX1], F8, isOutput=False)
    w1_d = nc.declare_dram_parameter("w1", [128, 2, 256], F8, isOutput=False)
    b1_d = nc.declare_dram_parameter("b1", [256, 1], F32, isOutput=False)
    w2b_d = nc.declare_dram_parameter("w2b", [2, 2, 128, NTB * 128], BF, isOutput=False)
    w2f_d = nc.declare_dram_parameter("w2f", [2, 128, TF8 * 2 * 128], F8, isOutput=False)
    b2_d = nc.declare_dram_parameter("b2", [256, 1], F32, isOutput=False)
    ws_d = nc.declare_dram_parameter("ws", [256, C * O], BF, isOutput=False)
    wcf_d = nc.declare_dram_parameter("wcf", [80, 2, 256], BF, isOutput=False)
    m80_d = nc.declare_dram_parameter("m80", [80, 2, C], F32, isOutput=False)
    m80b_d = nc.declare_dram_parameter("m80b", [80, 2, C], BF, isOutput=False)
    m580_d = nc.declare_dram_parameter("m580", [5, 80], F32, isOutput=False)
    maskg_d = nc.declare_dram_parameter("maskg", [2, 128, R * C], F32, isOutput=False)
    idf_d = nc.declare_dram_parameter("idf", [128, 128], F32, isOutput=False)
    idb_d = nc.declare_dram_parameter("idb", [128, 128], BF, isOutput=False)
    vout_d = nc.declare_dram_parameter("v_out", [B * C, O], F32, isOutput=True)

    with tile.TileContext(nc) as tc, ExitStack() as ctx:
        consts = ctx.enter_context(tc.tile_pool(name="consts", bufs=1))
        w1t = consts.tile([128, 2, 256], mybir.dt.float8e4, tag="w1t",
                          name="w1t")
        b1t = [consts.tile([128, 1], F32, tag=f"b1_{m}", name=f"b1_{m}") for m in range(2)]
        b2t = [consts.tile([128, 1], F32, tag=f"b2_{m}", name=f"b2_{m}") for m in range(2)]
        ws_t = [consts.tile([128, C * O], BF, tag=f"ws{m}", name=f"ws{m}") for m in range(2)]
        wcf = consts.tile([80, 2, 256], BF, tag="wcf", name="wcf")
        m80 = consts.tile([80, 2, C], F32, tag="m80", name="m80")
        m80b = consts.tile([80, 2, C], BF, tag="m80b", name="m80b")
        m580 = consts.tile([5, 80], F32, tag="m580", name="m580")
        idf = consts.tile([128, 128], F32, tag="idf", name="idf")
        idb = consts.tile([128, 128], BF, tag="idb", name="idb")
        maskg = [consts.tile([128, R * C], F32, tag=f"mg{m}", name=f"mg{m}")
                 for m in range(2)]
        # fast-inverse-sqrt magic seed (0x5f3759df) as an f32-bit pattern
        magic = consts.tile([128, 32], F32, tag="magic", name="magic")
        nc.vector.memset(
            magic, float(np.uint32(0x5F3759DF).view(np.float32)))
        magic5 = consts.tile([128, 5, 32], F32, tag="magic5", name="magic5")
        nc.vector.memset(
            magic5, float(np.uint32(0x5F3759DF).view(np.float32)))

        def dve_rsqrt(y, x, p, n, tmp_tag, iters=2):
            """y[:p,:n] = 1/sqrt(x[:p,:n]) on DVE only (bit trick +
            Newton).  No scalar engine -> no act-table thrash."""
            t = dpool.tile([128, 32], F32, tag=f"{tmp_tag}t", name=f"{tmp_tag}t")
            nc.vector.tensor_scalar(
                y.bitcast(mybir.dt.uint32),
                x.bitcast(mybir.dt.uint32), 1, None,
                op0=mybir.AluOpType.logical_shift_right)
            nc.vector.tensor_tensor(
                y.bitcast(mybir.dt.uint32),
                magic[:p, :n].bitcast(mybir.dt.uint32),
                y.bitcast(mybir.dt.uint32),
                op=mybir.AluOpType.subtract)
            for _ in range(iters):  # y *= 1.5 - 0.5*x*y*y
                nc.vector.tensor_mul(t[:p, :n], y, y)
                nc.vector.tensor_mul(t[:p, :n], t[:p, :n], x)
                nc.vector.tensor_scalar(
                    t[:p, :n], t[:p, :n], -0.5, 1.5,
                    op0=mybir.AluOpType.mult, op1=mybir.AluOpType.add)
                nc.vector.tensor_mul(y, y, t[:p, :n])

        # ---- persistent pools (whole-kernel lifetime, ring-buffered) ----
        h1pool = ctx.enter_context(tc.tile_pool(name="h1p", bufs=2))
        h8pool = ctx.enter_context(tc.tile_pool(name="h8p", bufs=2))
        impool = ctx.enter_context(tc.tile_pool(name="imp", bufs=2))
        w2pool = ctx.enter_context(tc.tile_pool(name="w2p", bufs=1))
        crawpool = ctx.enter_context(tc.tile_pool(name="crawp", bufs=2))
        capspool = ctx.enter_context(tc.tile_pool(name="capsp", bufs=2))
        rpool = ctx.enter_context(tc.tile_pool(name="rpool", bufs=2))
        dpool = ctx.enter_context(tc.tile_pool(name="dtmp", bufs=4))
        pmpool = ctx.enter_context(tc.tile_pool(name="pmp", bufs=5))
        cpsum = ctx.enter_context(tc.tile_pool(name="cpsum", bufs=4, space="PSUM"))
        tps = ctx.enter_context(tc.tile_pool(name="tps", bufs=2, space="PSUM"))
        rps = ctx.enter_context(tc.tile_pool(name="rps", bufs=2, space="PSUM"))

        w2bt = [[w2pool.tile([128, NTB, 128], BF, tag=f"w2b_{ig}_{og}",
                             name=f"w2b_{ig}_{og}")
                 for og in range(2)] for ig in range(2)]
        w2ft = [w2pool.tile([128, TF8, 2, 128], mybir.dt.float8e4,
                            tag=f"w2f_{og}", name=f"w2f_{og}")
                for og in range(2)]

        # ================= DMA issue block =================
        # Two HWDGE queues (sync + scalar); scalar's queue stays SHORT (5
        # early issues, no WAR waits) so its relu/exp compute never queues
        # behind DMA issues.  w1 + pixel-sliced im0 pieces lead on both
        # queues so conv1(0) can start ~11us in; w2 og0 pieces follow in
        # tap-consumption order (slice-precise dep tracking unblocks
        # conv2's taps as pieces land).  gpsimd/SWDGE takes the small
        # routing consts.
        for m in range(2):
            nc.gpsimd.dma_start(b1t[m], b1_d[m * 128:(m + 1) * 128, :])
        for m in range(2):
            nc.gpsimd.dma_start(b2t[m], b2_d[m * 128:(m + 1) * 128, :])
        nc.gpsimd.dma_start(idf, idf_d[:, :])
        nc.gpsimd.dma_start(idb, idb_d[:, :])
        for m in range(2):
            nc.gpsimd.dma_start(maskg[m], maskg_d[m])
        for m in range(2):
            nc.gpsimd.dma_start(ws_t[m], ws_d[m * 128:(m + 1) * 128, :])
        nc.gpsimd.dma_start(wcf, wcf_d[:, :, :])
        nc.gpsimd.dma_start(m80, m80_d[:, :, :])
        nc.gpsimd.dma_start(m80b, m80b_d[:, :, :])
        nc.gpsimd.dma_start(m580, m580_d[:, :])

        im = [None] * B

        def issue_im(b):
            imt = impool.tile([128, 2, NPIX1], mybir.dt.float8e4, tag="imA",
                              name="imA")
            nc.sync.dma_start(imt.rearrange("p j n -> p (j n)"),
                              x_d[b].rearrange("p j n -> p (j n)"))
            im[b] = imt

        def w2b_piece(eng, ig, og, t0, t1):
            a, bb = t0 - TF8, t1 - TF8
            eng.dma_start(
                w2bt[ig][og][:, a:bb].rearrange("p t m -> p (t m)"),
                w2b_d[ig, og, :, a * 128:bb * 128])

        PXS = 1568  # im0 pixel split point (3.5 of 7 conv1 chunks)
        im0 = impool.tile([128, 2, NPIX1], mybir.dt.float8e4, tag="imA",
                          name="imA")
        im[0] = im0
        # scalar queue: w1 + im0 front-low; sync: front-high + back halves
        # (front pieces on both queues so conv1's n=0 chunk lands first)
        nc.scalar.dma_start(w1t.rearrange("p j n -> p (j n)"),
                            w1_d.rearrange("p j n -> p (j n)"))
        nc.scalar.dma_start(im0[0:64, :, 0:PXS], x_d[0, 0:64, :, 0:PXS])
        w2b_piece(nc.scalar, 0, 0, TF8, 46)
        w2b_piece(nc.scalar, 1, 0, 46, 81)
        # sync queue: im0 pieces -> rest of og0 -> fp8 -> og1 -> im1-3
        nc.sync.dma_start(im0[64:128, :, 0:PXS], x_d[0, 64:128, :, 0:PXS])
        nc.sync.dma_start(im0[0:64, :, PXS:NPIX1], x_d[0, 0:64, :, PXS:NPIX1])
        nc.sync.dma_start(im0[64:128, :, PXS:NPIX1],
                          x_d[0, 64:128, :, PXS:NPIX1])
        w2b_piece(nc.sync, 1, 0, TF8, 46)
        w2b_piece(nc.sync, 0, 0, 46, 81)
        nc.sync.dma_start(w2ft[0].rearrange("p t j m -> p (t j m)"), w2f_d[0])
        issue_im(1)
        nc.sync.dma_start(w2ft[1].rearrange("p t j m -> p (t j m)"), w2f_d[1])
        w2b_piece(nc.sync, 0, 1, TF8, 46)
        w2b_piece(nc.sync, 1, 1, TF8, 46)
        w2b_piece(nc.sync, 0, 1, 46, 81)
        w2b_piece(nc.sync, 1, 1, 46, 81)
        issue_im(2)
        issue_im(3)

        # ================= stage pump =================
        pending = deque()
        tapctr = [0]
        STAGE_START, STAGE_EVERY = 16, 72

        def pump():
            tapctr[0] += 1
            if (pending and tapctr[0] >= STAGE_START
                    and (tapctr[0] - STAGE_START) % STAGE_EVERY == 0
                    and tapctr[0] >= pending[0][1]):
                pending.popleft()[0]()

        def drain():
            while pending:
                pending.popleft()[0]()

        # ================= per-image phases =================
        def conv1(b):
            # fp8 DoubleRow: K=256 (two 128-row k-groups) per instruction;
            # host pre-scales x by 16 and w1 by 256, undone by the relu
            # drain's 2^-12 activation scale.  n-outer so pixel chunks are
            # consumed in DMA arrival order.
            imt = im[b]
            h1t = h1pool.tile([128, 2, 56, 2, 28], BF, tag="h1t", name="h1t")
            for n in range(7):
                for m in range(2):
                    ps = cpsum.tile([128, 8, 56], F32, tag="cps", name="c1ps")
                    nc.tensor.matmul(ps, w1t[:, :, m * 128:(m + 1) * 128],
                                     imt[:, :, n * N1CH:(n + 1) * N1CH],
                                     start=True, stop=True,
                                     perf_mode=mybir.MatmulPerfMode.DoubleRow)
                    # single drain per (n, m): phase-interleave via strided
                    # view; alternate ACT/DVE so drains never pace conv1.
                    ps_v = ps.rearrange("p r (x2 ph) -> p r ph x2", ph=2)
                    dst = h1t[:, m, 8 * n:8 * n + 8, :, :]
                    if n % 2 == 0:
                        nc.scalar.activation(dst, ps_v, AF.Relu,
                                             bias=b1t[m], scale=2.0 ** -12)
                    else:
                        tmp = dpool.tile([128, 8, 2, 28], F32, tag="c1t",
                                         name="c1t")
                        nc.vector.tensor_scalar(
                            tmp, ps_v, 2.0 ** -12, b1t[m],
                            op0=mybir.AluOpType.mult,
                            op1=mybir.AluOpType.add)
                        nc.vector.tensor_scalar_max(dst, tmp, 0.0)
            return h1t

        def h1cast(h1t):
            # fp8 copy of h1 (x16) for the DoubleRow taps; one DVE op
            # (~3.6us), hidden under conv2's leading bf16 taps.  gpsimd
            # takes ~14ns/elem for fp8 stores -- keep it away from this.
            hf = h8pool.tile([128, 2, 56, 2, 28], mybir.dt.float8e4,
                             tag="h1f8", name="h1f8")
            nc.vector.tensor_scalar_mul(hf, h1t, 16.0)
            return hf

        def conv2(b, h1t, h1f8, craw):
            # 4 single-bank phases (og, y).  bf16 taps first (w2 x4096),
            # fp8 DoubleRow taps last (so the h1 cast hides under bf16);
            # both accumulate at the same 2^12 scale, undone in the drain.
            tapctr[0] = 0
            for og in range(2):
                for y in range(2):
                    ps = cpsum.tile([128, 288], F32, tag="cps", name="c2ps")
                    for t81 in range(TF8, 81):
                        kh, kw = t81 // 9, t81 % 9
                        for ig in range(2):
                            rhs = h1t[:, ig,
                                      kh + 24 * y:kh + 24 * y + 24:2,
                                      kw % 2, kw // 2:kw // 2 + 24]
                            nc.tensor.matmul(
                                ps, w2bt[ig][og][:, t81 - TF8, :], rhs,
                                start=(t81 == TF8 and ig == 0), stop=False)
                            pump()
                    for t81 in range(TF8):
                        kh, kw = t81 // 9, t81 % 9
                        rhs = h1f8[:, :,
                                   kh + 24 * y:kh + 24 * y + 24:2,
                                   kw % 2, kw // 2:kw // 2 + 24]
                        nc.tensor.matmul(
                            ps, w2ft[og][:, t81], rhs,
                            start=False, stop=(t81 == TF8 - 1),
                            perf_mode=mybir.MatmulPerfMode.DoubleRow)
                        pump()
                    # drain on DVE (keeps scalar's act table on Exp)
                    nc.vector.tensor_scalar(
                        craw[:, og, y * 288:(y + 1) * 288], ps,
                        2.0 ** -12, b2t[og],
                        op0=mybir.AluOpType.mult, op1=mybir.AluOpType.add)

        sqst = {}

        def squash_og0(b, craw):
            # og0 half of the squash front-end, pumped into conv2-b's og1
            # tap window (min_tap guards the og0 drain dependency).
            def f():
                pms = [pmpool.tile([128, 256], F32, tag="pm", name="pm")
                       for _ in PIX_CHUNKS]
                nsqs = dpool.tile([128, 5, R], F32, tag="nsqs", name="nsqs")
                for k, (p0, ln) in enumerate(PIX_CHUNKS):
                    tp = tps.tile([128, 128], F32, tag="tp", name="tp")
                    nc.tensor.transpose(tp[:ln, :], craw[:, 0, p0:p0 + ln],
                                        idf)
                    nc.scalar.activation(pms[k][:ln, 0:128], tp[:ln, :],
                                         AF.Copy)
                for k, (p0, ln) in enumerate(PIX_CHUNKS):
                    pm3 = pms[k].rearrange("p (r i) -> p r i", i=D)
                    sq = dpool.tile([128, 16, D], F32, tag="sqh", name="sqh")
                    eng = nc.gpsimd if k < 2 else nc.vector
                    eng.tensor_mul(sq[:ln], pm3[:ln, 0:16], pm3[:ln, 0:16])
                    nc.vector.reduce_sum(nsqs[:ln, k, 0:16], sq[:ln],
                                         axis=AX.X)
                sqst[b] = (pms, nsqs)
            return f

        def squash_finish(b, craw):
            # og1 half + packed scale chain + backward transposes.
            pms, nsqs = sqst.pop(b)
            capsbf = capspool.tile([128, 5, 256], BF, tag="cbf", name="cbf")
            capsT = capspool.tile([128, 2, PIX], BF, tag="cT", name="cT")
            capsum = capspool.tile([128, 2], F32, tag="csum", name="csum")
            for k, (p0, ln) in enumerate(PIX_CHUNKS):
                tp = tps.tile([128, 128], F32, tag="tp", name="tp")
                nc.tensor.transpose(tp[:ln, :], craw[:, 1, p0:p0 + ln], idf)
                nc.vector.tensor_copy(pms[k][:ln, 128:256], tp[:ln, :])
            for k, (p0, ln) in enumerate(PIX_CHUNKS):
                pm3 = pms[k].rearrange("p (r i) -> p r i", i=D)
                sq = dpool.tile([128, 16, D], F32, tag="sqh", name="sqh")
                eng = nc.gpsimd if k < 2 else nc.vector
                eng.tensor_mul(sq[:ln], pm3[:ln, 16:32], pm3[:ln, 16:32])
                nc.vector.reduce_sum(nsqs[:ln, k, 16:32], sq[:ln],
                                     axis=AX.X)
            # scale = n * rsqrt(n) / (1+n), all on DVE, chunk-packed
            sqas = dpool.tile([128, 5, R], F32, tag="sqas", name="sqas")
            rys = dpool.tile([128, 5, R], F32, tag="rys", name="rys")
            nc.vector.tensor_scalar(
                rys.bitcast(mybir.dt.uint32), nsqs.bitcast(mybir.dt.uint32),
                1, None, op0=mybir.AluOpType.logical_shift_right)
            nc.vector.tensor_tensor(
                rys.bitcast(mybir.dt.uint32),
                magic5.bitcast(mybir.dt.uint32),
                rys.bitcast(mybir.dt.uint32), op=mybir.AluOpType.subtract)
            nt = dpool.tile([128, 5, R], F32, tag="nt", name="nt")
            nc.vector.tensor_mul(nt, rys, rys)
            nc.vector.tensor_mul(nt, nt, nsqs)
            nc.vector.tensor_scalar(
                nt, nt, -0.5, 1.5,
                op0=mybir.AluOpType.mult, op1=mybir.AluOpType.add)
            nc.vector.tensor_mul(rys, rys, nt)
            nc.vector.tensor_scalar_add(sqas, nsqs, 1.0)
            nc.vector.reciprocal(sqas, sqas)
            nc.vector.tensor_mul(sqas, sqas, rys)
            nc.vector.tensor_mul(sqas, nsqs, sqas)
            for k, (p0, ln) in enumerate(PIX_CHUNKS):
                pm3 = pms[k].rearrange("p (r i) -> p r i", i=D)
                cbf3 = capsbf[:, k].rearrange("p (r i) -> p r i", i=D)
                eng = nc.gpsimd if k < 2 else nc.vector
                eng.tensor_mul(
                    cbf3[:ln], pm3[:ln],
                    sqas[:ln, k].unsqueeze(2).broadcast_to([ln, R, D]))
            for k, (p0, ln) in enumerate(PIX_CHUNKS):
                for og in range(2):
                    tb = tps.tile([128, 128], BF, tag="tp", name="tb")
                    nc.tensor.transpose(
                        tb[:, :ln],
                        capsbf[:ln, k, og * 128:(og + 1) * 128],
                        idb[:ln, :ln])
                    if og == 0:
                        nc.scalar.activation(capsT[:, og, p0:p0 + ln],
                                             tb[:, :ln], AF.Copy)
                    else:
                        nc.vector.tensor_copy(capsT[:, og, p0:p0 + ln],
                                              tb[:, :ln])
            for g in range(2):
                nc.vector.reduce_sum(capsum[:, g:g + 1], capsT[:, g],
                                     axis=AX.X)
            return capsbf, capsT, capsum

        # ================= routing (per image, staged) =================
        def register_routing(b, capsbf, capsT, capsum):
            st = {}
            blog = rpool.tile([128, 5, R, C], F32, tag="blog", name="blog")
            # chunk 4 covers only 64 partitions; zero the rest once so the
            # fused whole-tile exp/reduce stay finite there.
            nc.gpsimd.memset(blog[64:128, 4], 0.0)

            def sv_t4(it, last=False):
                # (c,o)-packed layout: partition p = c_local*16+o, halves
                # h=0 (c 0-4) / h=1 (c 5-9).  The per-class s/T4 matmuls
                # batch into 4 + 4 wide ones; the c==c' diagonal is pulled
                # out with a mask multiply + reduce on DVE.
                def f():
                    sF = dpool.tile([80, 2], F32, tag="sF", name="sF")
                    if it == 0:
                        csb = dpool.tile([128, 2], BF, tag="csb", name="csb")
                        nc.vector.tensor_scalar_mul(csb, capsum, 1.0 / C)
                        sps = rps.tile([80, 2], F32, tag="rps", name="sps0")
                        for h in range(2):
                            for m in range(2):
                                nc.tensor.matmul(
                                    sps[:, h:h + 1],
                                    ws_t[m][:, 80 * h:80 * (h + 1)],
                                    csb[:, m:m + 1],
                                    start=(m == 0), stop=(m == 1))
                        nc.vector.tensor_copy(sF, sps)
                    else:
                        Gp = st['Gp']
                        for h in range(2):
                            sps = rps.tile([80, C], F32, tag="rps",
                                           name=f"sps{h}")
                            for m in range(2):
                                nc.tensor.matmul(
                                    sps, ws_t[m][:, 80 * h:80 * (h + 1)],
                                    Gp[m], start=(m == 0), stop=(m == 1))
                            fm = dpool.tile([80, C], F32, tag="sfm",
                                            name="sfm")
                            nc.vector.tensor_mul(fm, sps, m80[:, h])
                            nc.vector.reduce_sum(sF[:, h:h + 1], fm,
                                                 axis=AX.X)
                    # squash on the packed layout: per-class norms via a
                    # block-ones matmul, scale chain on [5, 2], broadcast
                    # back via a K=5 matmul.  All elementwise on DVE.
                    sq2 = dpool.tile([80, 2], F32, tag="sq2", name="sq2")
                    nc.vector.tensor_mul(sq2, sF, sF)
                    n2ps = rps.tile([5, 2], F32, tag="rps", name="n2ps")
                    for h in range(2):
                        nc.tensor.matmul(n2ps[:, h:h + 1], m80[:, 0, :5],
                                         sq2[:, h:h + 1],
                                         start=True, stop=True)
                    n2 = dpool.tile([5, 2], F32, tag="n2", name="n2")
                    nc.vector.tensor_copy(n2, n2ps)
                    ry = dpool.tile([128, 16], F32, tag="ry", name="ry")
                    dve_rsqrt(ry[:5, :2], n2, 5, 2, "vr", iters=1)
                    a2 = dpool.tile([5, 2], F32, tag="a2", name="a2")
                    nc.vector.tensor_scalar_add(a2, n2, 1.0)
                    nc.vector.reciprocal(a2, a2)
                    nc.vector.tensor_mul(a2, a2, ry[:5, :2])
                    nc.vector.tensor_mul(a2, n2, a2)
                    scps = rps.tile([80, 2], F32, tag="rps", name="scps")
                    for h in range(2):
                        nc.tensor.matmul(scps[:, h:h + 1], m580,
                                         a2[:, h:h + 1],
                                         start=True, stop=True)
                    if last:
                        vff = dpool.tile([80, 2], F32, tag="vff", name="vff")
                        nc.vector.tensor_mul(vff, sF, scps)
                        vT = rps.tile([2, 80], F32, tag="rps", name="vT")
                        nc.tensor.transpose(vT, vff, idf[:80, :80])
                        vout = rpool.tile([2, 80], F32, tag="vout",
                                          name="vout")
                        nc.vector.tensor_copy(vout, vT)
                        nc.sync.dma_start(
                            vout_d[b * C:(b + 1) * C, :]
                            .rearrange("(h f) o -> h (f o)", f=5), vout)
                        return
                    vF2 = rpool.tile([80, 2], BF, tag="vF2", name="vF2")
                    nc.vector.tensor_mul(vF2, sF, scps)
                    # T4: rhs = vF broadcast masked to [80, C] per half,
                    # contraction over the packed (c,o) dim in 2 halves.
                    T4 = [rpool.tile([128, R * C], BF, tag=f"T4_{m}",
                                     name=f"T4_{m}") for m in range(2)]
                    vm = dpool.tile([80, 2, C], BF, tag="vm", name="vm")
                    nc.vector.tensor_mul(
                        vm, vF2.unsqueeze(2).broadcast_to([80, 2, C]), m80b)
                    for m in range(2):
                        t4 = rps.tile([128, C], F32, tag="rps", name="t4")
                        for h in range(2):
                            nc.tensor.matmul(
                                t4, wcf[:, h, m * 128:(m + 1) * 128],
                                vm[:, h], start=(h == 0), stop=(h == 1))
                        data = t4.unsqueeze(1).broadcast_to([128, R, C])
                        mk = maskg[m].rearrange("p (r c) -> p r c", c=C)
                        nc.vector.tensor_mul(
                            T4[m].rearrange("p (r c) -> p r c", c=C),
                            data, mk)
                    st['T4'] = T4
                return f

            def dlstage(it):
                def f():
                    T4 = st['T4']
                    for k, (p0, ln) in enumerate(PIX_CHUNKS):
                        dl = rps.tile([128, R, C], F32, tag="rps", name="dl")
                        for kc in range(2):
                            nc.tensor.matmul(
                                dl[:ln], capsT[:, kc, p0:p0 + ln],
                                T4[kc], start=(kc == 0), stop=(kc == 1))
                        if it == 0:
                            nc.vector.tensor_copy(blog[:ln, k], dl[:ln])
                        else:
                            nc.vector.tensor_add(blog[:ln, k], blog[:ln, k],
                                                 dl[:ln])
                return f

            def efstage():
                # fused softmax-weight + F' stage: whole-tile exp/reduce/
                # reciprocal, 2-way split weighting, then the 5x2 F4
                # accumulation matmuls.
                e = rpool.tile([128, 5, R, C], BF, tag="e", name="e")
                cwt = rpool.tile([128, 5, R, D], BF, tag="cw", name="cw")
                F4 = [rps.tile([128, R * C], F32, tag="rps", name=f"F4_{m}")
                      for m in range(2)]
                dens = dpool.tile([128, 5, R], F32, tag="dens", name="dens")
                nc.scalar.activation(e, blog, AF.Exp)
                nc.vector.reduce_sum(dens, e, axis=AX.X)
                nc.vector.reciprocal(dens, dens)
                cbf4 = capsbf.rearrange("p k (r i) -> p k r i", i=D)
                nc.gpsimd.tensor_mul(
                    cwt[:, 0:2], cbf4[:, 0:2],
                    dens[:, 0:2].unsqueeze(3).broadcast_to([128, 2, R, D]))
                nc.vector.tensor_mul(
                    cwt[:, 2:5], cbf4[:, 2:5],
                    dens[:, 2:5].unsqueeze(3).broadcast_to([128, 3, R, D]))
                for k, (p0, ln) in enumerate(PIX_CHUNKS):
                    cwf = cwt[:, k].rearrange("p r i -> p (r i)")
                    ef = e[:, k].rearrange("p r c -> p (r c)")
                    for m in range(2):
                        nc.tensor.matmul(F4[m],
                                         cwf[:ln, m * 128:(m + 1) * 128],
                                         ef[:ln],
                                         start=(k == 0), stop=(k == 4))
                Gp = [rpool.tile([128, C], BF, tag=f"G{m}", name=f"G{m}")
                      for m in range(2)]
                for m in range(2):
                    fm = dpool.tile([128, R * C], BF, tag="fm", name="fm")
                    nc.vector.tensor_mul(fm, F4[m], maskg[m])
                    gf = dpool.tile([128, C], F32, tag="gf", name="gf")
                    nc.vector.reduce_sum(
                        gf, fm.rearrange("p (r c) -> p c r", c=C), axis=AX.X)
                    nc.vector.tensor_copy(Gp[m], gf)
                st['Gp'] = Gp

            pending.extend([
                (sv_t4(0), 0), (dlstage(0), 0), (efstage, 0),
                (sv_t4(1), 0), (dlstage(1), 0), (efstage, 0),
                (sv_t4(2, last=True), 0),
            ])

        # ================= main pipeline =================
        h1s = {0: conv1(0)}
        for b in range(B):
            craw = crawpool.tile([128, 2, PIX], F32, tag="craw", name="craw")
            hf = h1cast(h1s[b])
            pending.append((squash_og0(b, craw), 300))
            conv2(b, h1s[b], hf, craw)
            if b + 1 < B:
                h1s[b + 1] = conv1(b + 1)
            drain()  # safety: all pumped stages must be fully emitted
            cbs = squash_finish(b, craw)
            register_routing(b, *cbs)
        drain()  # routing of the last image (exposed tail)

    nc.compile()
    return nc


@functools.lru_cache(maxsize=1)
def _get_nc():
    return _build_nc()


def _prep_consts(conv1_w, conv1_b, conv2_w, conv2_b, route_w):
    bf = ml_dtypes.bfloat16
    f8 = ml_dtypes.float8_e4m3
    f32 = np.float32
    w1 = np.zeros((256, 256), f32)
    w1[:K1] = conv1_w.astype(f32).transpose(1, 2, 3, 0).reshape(K1, 256)
    # DoubleRow lhsT layout [p, j, m] = w1[j*128+p, m], fp8 with x256 scale
    w1dr = np.clip(w1 * 256.0, -240, 240).reshape(2, 128, 256).transpose(1, 0, 2)
    w2 = conv2_w.astype(f32).reshape(2, 128, 2, 128, 81)  # [og, mo, ig, ki, tap]
    # bf16 taps TF8..80: [ig, og, ki, tap, mo], x4096 (exact pow2) so they
    # accumulate at the same scale as the fp8 (x16 * x256) taps
    w2b = (w2[..., TF8:].transpose(2, 0, 3, 4, 1) * 4096.0)
    # fp8 taps 0..TF8-1: [og, ki, tap, ig, mo], x256
    w2f = np.clip(w2[..., :TF8].transpose(0, 3, 4, 2, 1) * 256.0, -240, 240)
    ws = route_w.astype(f32).transpose(0, 2, 1, 3).reshape(256, C * O)
    # wcf[(c_l,o), h, q] = wcob[o, 5h+c_l, q]; wcob[o,c,q] = route_w view
    wcob = route_w.astype(f32).transpose(3, 1, 0, 2).reshape(O, C, 256)
    wcf = np.zeros((80, 2, 256), f32)
    for cl in range(5):
        for o in range(O):
            for h in range(2):
                wcf[cl * 16 + o, h] = wcob[o, 5 * h + cl]
    # m80[p=(c_l,o), h, c'] = (c' == 5h + c_l)
    m80 = np.zeros((80, 2, C), f32)
    for cl in range(5):
        for h in range(2):
            m80[cl * 16:(cl + 1) * 16, h, 5 * h + cl] = 1.0
    # m580[j, p] = (j == p//16)
    m580 = np.zeros((5, 80), f32)
    for j in range(5):
        m580[j, j * 16:(j + 1) * 16] = 1.0
    maskg = np.zeros((2, 128, R * C), f32)
    for m in range(2):
        for j in range(128):
            r = m * 16 + j // D
            maskg[m, j, r * C:(r + 1) * C] = 1.0
    return {
        "w1": np.ascontiguousarray(w1dr).astype(f8),
        "b1": np.ascontiguousarray(conv1_b.astype(f32).reshape(256, 1)),
        "w2b": np.ascontiguousarray(w2b).reshape(2, 2, 128, NTB * 128).astype(bf),
        "w2f": np.ascontiguousarray(w2f).reshape(2, 128, TF8 * 2 * 128).astype(f8),
        "b2": np.ascontiguousarray(conv2_b.astype(f32).reshape(256, 1)),
        "ws": np.ascontiguousarray(ws).astype(bf),
        "wcf": np.ascontiguousarray(wcf).astype(bf),
        "m80": m80,
        "m80b": np.ascontiguousarray(m80).astype(bf),
        "m580": m580,
        "idf": np.eye(128, dtype=f32),
        "idb": np.eye(128, dtype=f32).astype(bf),
        "maskg": maskg,
    }


def _ensure_ntff_hook():
    """The agent image's antenv lacks axon_hooks; shim it so trace=True works."""
    import sys
    import types
    try:
        from antenv import axon_hooks  # noqa: F401
        return
    except ImportError:
        pass
    mod = types.ModuleType("antenv.axon_hooks")
    _h = [None]
    mod.get_axon_ntff_profile_hook = lambda: _h[0]
    mod.set_axon_ntff_profile_hook = lambda h: _h.__setitem__(0, h)
    sys.modules["antenv.axon_hooks"] = mod
    try:
        from trn_agent_boot.trn_boot import _ntff_profile_via_ctypes
        mod.set_axon_ntff_profile_hook(
            _ntff_profile_via_ctypes("/opt/axon/libaxon_pjrt.so"))
    except Exception as e:  # degrade: trace skipped, run still works
        print(f"ntff hook shim failed: {e}")


def run(x, conv1_w, conv1_b, conv2_w, conv2_b, route_w, trace=False, cores=NCORES):
    if trace:
        _ensure_ntff_hook()
    x = np.asarray(x, np.float32)
    nb = x.shape[0]
    consts = _prep_consts(np.asarray(conv1_w), np.asarray(conv1_b),
                          np.asarray(conv2_w), np.asarray(conv2_b),
                          np.asarray(route_w))
    win = np.lib.stride_tricks.sliding_window_view(x, (9, 9), axis=(2, 3))
    xb = (win.transpose(0, 1, 4, 5, 2, 3)          # [b, c, kh, kw, y, x]
          .reshape(nb, K1, NPIX1))
    # fp8 DoubleRow layout [b, p, j, n], rows 243..255 zero, x16 scale
    xq = np.zeros((nb, 256, NPIX1), np.float32)
    xq[:, :K1] = np.clip(xb * 16.0, -240, 240)
    xq = np.ascontiguousarray(
        xq.reshape(nb, 2, 128, NPIX1).transpose(0, 2, 1, 3)
    ).astype(ml_dtypes.float8_e4m3)
    assert nb == B * cores
    in_maps = []
    for cid in range(cores):
        m = dict(consts)
        m["x"] = np.ascontiguousarray(xq[cid * B:(cid + 1) * B])
        in_maps.append(m)
    res = run_bass_kernel_spmd(_get_nc(), in_maps, list(range(cores)), trace=trace)
    out = np.concatenate([r["v_out"].reshape(B, C, O) for r in res.results], axis=0)
    return out.astype(np.float32), res


def kernel(x, conv1_w, conv1_b, conv2_w, conv2_b, route_w):
    out, _ = run(x, conv1_w, conv1_b, conv2_w, conv2_b, route_w, trace=False)
    return out


# revision 11
# speedup vs baseline: 1.4388x; 1.0543x over previous
# CapsuleNetwork Trainium2 kernel (8-core data parallel, 4 images/core).
#
# Per core, fully software-pipelined over images:
#   conv1 3->256 k9 s1 (im2col K=243, fp8 DoubleRow) -> conv2 256->256 k9 s2
#   (81-tap PSUM accumulation, hybrid bf16/fp8) -> squash -> 3-iter routing.
# conv2 runs as 4 (og, y) phases of one PSUM bank each; taps 14..80 run in
# bf16 (w2 host-scaled x4096, exact pow2), taps 0..13 run in fp8 DoubleRow
# (h1 cast x16 on DVE/gpsimd, w2 x256) -- the tap split keeps the final
# rel-err ~0.015 while shaving ~25% of conv2's PE cycles.  Image b's routing
# instructions are interleaved ("pumped") into image b+1's conv2 tap loop so
# the vector/scalar-bound routing hides under the tensor-bound conv2 stream.
# Startup: w1 + the first im2col ride short pixel-sliced pieces on both HWDGE
# queues so conv1(0) starts ~7us earlier; conv1 walks pixel-chunks n-outer to
# consume them in arrival order.  All squash/routing elementwise math runs on
# DVE/gpsimd (single activation table load); softmax/F' stages are fused into
# whole-tile ops to cut the exposed routing tail of the last image.
import functools
from collections import deque
from contextlib import ExitStack

import numpy as np
import ml_dtypes

import concourse.bass as bass
import concourse.tile as tile
from concourse import bacc
from concourse import mybir
from concourse.bass_utils import run_bass_kernel_spmd

BF = mybir.dt.bfloat16
F32 = mybir.dt.float32
AF = mybir.ActivationFunctionType
AX = mybir.AxisListType

NCORES = 8
B = 4              # images per core
K1 = 243           # 3*9*9 im2col contraction
NPIX1 = 3136       # 56*56 conv1 output pixels
N1CH = 448         # conv1 moving chunk (3136 = 7*448)
PIX = 576          # 24*24 conv2 output pixels
PIX_CHUNKS = [(0, 128), (128, 128), (256, 128), (384, 128), (512, 64)]
R, D, C, O = 32, 8, 10, 16
TF8 = 14           # conv2 taps 0..TF8-1 in fp8 DoubleRow, rest bf16
NTB = 81 - TF8     # bf16 taps


def _build_nc():
    nc = bacc.Bacc("TRN2", target_bir_lowering=False, debug=False)
    F8 = mybir.dt.float8e4
    x_d = nc.declare_dram_parameter("x", [B, 128, 2, NPIX1], F8, isOutput=False)
    w1_d = nc.declare_dram_parameter("w1", [128, 2, 256], F8, isOutput=False)
    b1_d = nc.declare_dram_parameter("b1", [256, 1], F32, isOutput=False)
    w2b_d = nc.declare_dram_parameter("w2b", [2, 2, 128, NTB * 128], BF, isOutput=False)
    w2f_d = nc.declare_dram_parameter("w2f", [2, 128, TF8 * 2 * 128], F8, isOutput=False)
    b2_d = nc.declare_dram_parameter("b2", [256, 1], F32, isOutput=False)
    ws_d = nc.declare_dram_parameter("ws", [256, C * O], BF, isOutput=False)
    wcf_d = nc.declare_dram_parameter("wcf", [80, 2, 256], BF, isOutput=False)
    m80_d = nc.declare_dram_parameter("m80", [80, 2, C], F32, isOutput=False)
    m80b_d = nc.declare_dram_parameter("m80b", [80, 2, C], BF, isOutput=False)
    m580_d = nc.declare_dram_parameter("m580", [5, 80], F32, isOutput=False)
    maskg_d = nc.declare_dram_parameter("maskg", [2, 128, R * C], F32, isOutput=False)
    idf_d = nc.declare_dram_parameter("idf", [128, 128], F32, isOutput=False)
    idb_d = nc.declare_dram_parameter("idb", [128, 128], BF, isOutput=False)
    vout_d = nc.declare_dram_parameter("v_out", [B * C, O], F32, isOutput=True)

    with tile.TileContext(nc) as tc, ExitStack() as ctx:
        consts = ctx.enter_context(tc.tile_pool(name="consts", bufs=1))
        w1t = consts.tile([128, 2, 256], mybir.dt.float8e4, tag="w1t",
                          name="w1t")
        b1t = [consts.tile([128, 1], F32, tag=f"b1_{m}", name=f"b1_{m}") for m in range(2)]
        b2t = [consts.tile([128, 1], F32, tag=f"b2_{m}", name=f"b2_{m}") for m in range(2)]
        ws_t = [consts.tile([128, C * O], BF, tag=f"ws{m}", name=f"ws{m}") for m in range(2)]
        wcf = consts.tile([80, 2, 256], BF, tag="wcf", name="wcf")
        m80 = consts.tile([80, 2, C], F32, tag="m80", name="m80")
        m80b = consts.tile([80, 2, C], BF, tag="m80b", name="m80b")
        m580 = consts.tile([5, 80], F32, tag="m580", name="m580")
        idf = consts.tile([128, 128], F32, tag="idf", name="idf")
        idb = consts.tile([128, 128], BF, tag="idb", name="idb")
        maskg = [consts.tile([128, R * C], F32, tag=f"mg{m}", name=f"mg{m}")
                 for m in range(2)]
        # fast-inverse-sqrt magic seed (0x5f3759df) as an f32-bit pattern
        magic = consts.tile([128, 32], F32, tag="magic", name="magic")
        nc.vector.memset(
            magic, float(np.uint32(0x5F3759DF).view(np.float32)))
        magic5 = consts.tile([128, 5, 32], F32, tag="magic5", name="magic5")
        nc.vector.memset(
            magic5, float(np.uint32(0x5F3759DF).view(np.float32)))

        def dve_rsqrt(y, x, p, n, tmp_tag, iters=2):
            """y[:p,:n] = 1/sqrt(x[:p,:n]) on DVE only (bit trick +
            Newton).  No scalar engine -> no act-table thrash."""
            t = dpool.tile([128, 32], F32, tag=f"{tmp_tag}t", name=f"{tmp_tag}t")
            nc.vector.tensor_scalar(
                y.bitcast(mybir.dt.uint32),
                x.bitcast(mybir.dt.uint32), 1, None,
                op0=mybir.AluOpType.logical_shift_right)
            nc.vector.tensor_tensor(
                y.bitcast(mybir.dt.uint32),
                magic[:p, :n].bitcast(mybir.dt.uint32),
                y.bitcast(mybir.dt.uint32),
                op=mybir.AluOpType.subtract)
            for _ in range(iters):  # y *= 1.5 - 0.5*x*y*y
                nc.vector.tensor_mul(t[:p, :n], y, y)
                nc.vector.tensor_mul(t[:p, :n], t[:p, :n], x)
                nc.vector.tensor_scalar(
                    t[:p, :n], t[:p, :n], -0.5, 1.5,
                    op0=mybir.AluOpType.mult, op1=mybir.AluOpType.add)
                nc.vector.tensor_mul(y, y, t[:p, :n])

        # ---- persistent pools (whole-kernel lifetime, ring-buffered) ----
        h1pool = ctx.enter_context(tc.tile_pool(name="h1p", bufs=2))
        h8pool = ctx.enter_context(tc.tile_pool(name="h8p", bufs=2))
        impool = ctx.enter_context(tc.tile_pool(name="imp", bufs=2))
        w2pool = ctx.enter_context(tc.tile_pool(name="w2p", bufs=1))
        crawpool = ctx.enter_context(tc.tile_pool(name="crawp", bufs=2))
        capspool = ctx.enter_context(tc.tile_pool(name="capsp", bufs=2))
        rpool = ctx.enter_context(tc.tile_pool(name="rpool", bufs=2))
        dpool = ctx.enter_context(tc.tile_pool(name="dtmp", bufs=4))
        pmpool = ctx.enter_context(tc.tile_pool(name="pmp", bufs=5))
        cpsum = ctx.enter_context(tc.tile_pool(name="cpsum", bufs=4, space="PSUM"))
        tps = ctx.enter_context(tc.tile_pool(name="tps", bufs=2, space="PSUM"))
        rps = ctx.enter_context(tc.tile_pool(name="rps", bufs=2, space="PSUM"))

        w2bt = [[w2pool.tile([128, NTB, 128], BF, tag=f"w2b_{ig}_{og}",
                             name=f"w2b_{ig}_{og}")
                 for og in range(2)] for ig in range(2)]
        w2ft = [w2pool.tile([128, TF8, 2, 128], mybir.dt.float8e4,
                            tag=f"w2f_{og}", name=f"w2f_{og}")
                for og in range(2)]

        # ================= DMA issue block =================
        # Two HWDGE queues (sync + scalar); scalar's queue stays SHORT (5
        # early issues, no WAR waits) so its relu/exp compute never queues
        # behind DMA issues.  w1 + pixel-sliced im0 pieces lead on both
        # queues so conv1(0) can start ~11us in; w2 og0 pieces follow in
        # tap-consumption order (slice-precise dep tracking unblocks
        # conv2's taps as pieces land).  gpsimd/SWDGE takes the small
        # routing consts.
        for m in range(2):
            nc.gpsimd.dma_start(b1t[m], b1_d[m * 128:(m + 1) * 128, :])
        for m in range(2):
            nc.gpsimd.dma_start(b2t[m], b2_d[m * 128:(m + 1) * 128, :])
        nc.gpsimd.dma_start(idf, idf_d[:, :])
        nc.gpsimd.dma_start(idb, idb_d[:, :])
        for m in range(2):
            nc.gpsimd.dma_start(maskg[m], maskg_d[m])
        for m in range(2):
            nc.gpsimd.dma_start(ws_t[m], ws_d[m * 128:(m + 1) * 128, :])
        nc.gpsimd.dma_start(wcf, wcf_d[:, :, :])
        nc.gpsimd.dma_start(m80, m80_d[:, :, :])
        nc.gpsimd.dma_start(m80b, m80b_d[:, :, :])
        nc.gpsimd.dma_start(m580, m580_d[:, :])

        im = [None] * B

        def issue_im(b):
            imt = impool.tile([128, 2, NPIX1], mybir.dt.float8e4, tag="imA",
                              name="imA")
            nc.sync.dma_start(imt.rearrange("p j n -> p (j n)"),
                              x_d[b].rearrange("p j n -> p (j n)"))
            im[b] = imt

        def w2b_piece(eng, ig, og, t0, t1):
            a, bb = t0 - TF8, t1 - TF8
            eng.dma_start(
                w2bt[ig][og][:, a:bb].rearrange("p t m -> p (t m)"),
                w2b_d[ig, og, :, a * 128:bb * 128])

        PXS = 1568  # im0 pixel split point (3.5 of 7 conv1 chunks)
        im0 = impool.tile([128, 2, NPIX1], mybir.dt.float8e4, tag="imA",
                          name="imA")
        im[0] = im0
        # scalar queue: w1 + im0 front-low + og1 (consumed first) pieces
        # (front im0 pieces on both queues so conv1's n=0 chunk lands first)
        nc.scalar.dma_start(w1t.rearrange("p j n -> p (j n)"),
                            w1_d.rearrange("p j n -> p (j n)"))
        nc.scalar.dma_start(im0[0:64, :, 0:PXS], x_d[0, 0:64, :, 0:PXS])
        w2b_piece(nc.scalar, 0, 1, TF8, 28)
        w2b_piece(nc.scalar, 0, 1, 28, 46)
        w2b_piece(nc.scalar, 1, 1, 46, 64)
        w2b_piece(nc.scalar, 1, 1, 64, 81)
        w2b_piece(nc.scalar, 0, 0, TF8, 46)
        # sync queue: im0 pieces -> rest of og1 -> fp8 -> og0 -> im1-3
        nc.sync.dma_start(im0[64:128, :, 0:PXS], x_d[0, 64:128, :, 0:PXS])
        nc.sync.dma_start(im0[0:64, :, PXS:NPIX1], x_d[0, 0:64, :, PXS:NPIX1])
        nc.sync.dma_start(im0[64:128, :, PXS:NPIX1],
                          x_d[0, 64:128, :, PXS:NPIX1])
        w2b_piece(nc.sync, 1, 1, TF8, 28)
        w2b_piece(nc.sync, 1, 1, 28, 46)
        w2b_piece(nc.sync, 0, 1, 46, 81)
        nc.sync.dma_start(w2ft[1].rearrange("p t j m -> p (t j m)"), w2f_d[1])
        issue_im(1)
        nc.sync.dma_start(w2ft[0].rearrange("p t j m -> p (t j m)"), w2f_d[0])
        w2b_piece(nc.sync, 1, 0, TF8, 46)
        w2b_piece(nc.sync, 0, 0, 46, 81)
        w2b_piece(nc.sync, 1, 0, 46, 81)
        issue_im(2)
        issue_im(3)

        # ================= stage pump =================
        pending = deque()
        tapctr = [0]
        STAGE_START, STAGE_EVERY = 16, 56

        def pump():
            tapctr[0] += 1
            if (pending and tapctr[0] >= STAGE_START
                    and (tapctr[0] - STAGE_START) % STAGE_EVERY == 0
                    and tapctr[0] >= pending[0][1]):
                pending.popleft()[0]()

        def drain():
            while pending:
                pending.popleft()[0]()

        # ================= per-image phases =================
        def conv1(b):
            # fp8 DoubleRow: K=256 (two 128-row k-groups) per instruction;
            # host pre-scales x by 16 and w1 by 256, undone by the relu
            # drain's 2^-12 activation scale.  n-outer so pixel chunks are
            # consumed in DMA arrival order.
            imt = im[b]
            h1t = h1pool.tile([128, 2, 56, 2, 28], BF, tag="h1t", name="h1t")
            for n in range(7):
                for m in range(2):
                    ps = cpsum.tile([128, 8, 56], F32, tag="cps", name="c1ps")
                    nc.tensor.matmul(ps, w1t[:, :, m * 128:(m + 1) * 128],
                                     imt[:, :, n * N1CH:(n + 1) * N1CH],
                                     start=True, stop=True,
                                     perf_mode=mybir.MatmulPerfMode.DoubleRow)
                    # single drain per (n, m): phase-interleave via strided
                    # view; alternate ACT/DVE so drains never pace conv1.
                    ps_v = ps.rearrange("p r (x2 ph) -> p r ph x2", ph=2)
                    dst = h1t[:, m, 8 * n:8 * n + 8, :, :]
                    if n % 2 == 0:
                        nc.scalar.activation(dst, ps_v, AF.Relu,
                                             bias=b1t[m], scale=2.0 ** -12)
                    else:
                        tmp = dpool.tile([128, 8, 2, 28], F32, tag="c1t",
                                         name="c1t")
                        nc.vector.tensor_scalar(
                            tmp, ps_v, 2.0 ** -12, b1t[m],
                            op0=mybir.AluOpType.mult,
                            op1=mybir.AluOpType.add)
                        nc.vector.tensor_scalar_max(dst, tmp, 0.0)
            return h1t

        def h1cast(h1t):
            # fp8 copy of h1 (x16) for the DoubleRow taps; one DVE op
            # (~3.6us), hidden under conv2's leading bf16 taps.  gpsimd
            # takes ~14ns/elem for fp8 stores -- keep it away from this.
            hf = h8pool.tile([128, 2, 56, 2, 28], mybir.dt.float8e4,
                             tag="h1f8", name="h1f8")
            nc.vector.tensor_scalar_mul(hf, h1t, 16.0)
            return hf

        def conv2(b, h1t, h1f8, craw):
            # 4 single-bank phases (og, y), og1 FIRST so og1's full squash
            # half can pump into og0's tap windows.  bf16 taps first (w2
            # x4096), fp8 DoubleRow taps last (so the h1 cast hides under
            # bf16); both accumulate at the same 2^12 scale, undone in the
            # drain.
            tapctr[0] = 0
            for og in (1, 0):
                for y in range(2):
                    ps = cpsum.tile([128, 288], F32, tag="cps", name="c2ps")
                    for t81 in range(TF8, 81):
                        kh, kw = t81 // 9, t81 % 9
                        for ig in range(2):
                            rhs = h1t[:, ig,
                                      kh + 24 * y:kh + 24 * y + 24:2,
                                      kw % 2, kw // 2:kw // 2 + 24]
                            nc.tensor.matmul(
                                ps, w2bt[ig][og][:, t81 - TF8, :], rhs,
                                start=(t81 == TF8 and ig == 0), stop=False)
                            pump()
                    for t81 in range(TF8):
                        kh, kw = t81 // 9, t81 % 9
                        rhs = h1f8[:, :,
                                   kh + 24 * y:kh + 24 * y + 24:2,
                                   kw % 2, kw // 2:kw // 2 + 24]
                        nc.tensor.matmul(
                            ps, w2ft[og][:, t81], rhs,
                            start=False, stop=(t81 == TF8 - 1),
                            perf_mode=mybir.MatmulPerfMode.DoubleRow)
                        pump()
                    # drain on DVE (keeps scalar's act table on Exp)
                    nc.vector.tensor_scalar(
                        craw[:, og, y * 288:(y + 1) * 288], ps,
                        2.0 ** -12, b2t[og],
                        op0=mybir.AluOpType.mult, op1=mybir.AluOpType.add)

        sqst = {}

        def scale_half(pms, nsqs, capsbf, r0, r1):
            # scale = n * rsqrt(n) / (1+n) for routes r0:r1, all on DVE,
            # chunk-packed; then scale the caps into capsbf (DVE/gpsimd).
            rn = r1 - r0
            nh = nsqs[:, :, r0:r1]
            sqas = dpool.tile([128, 5, 16], F32, tag="sqas", name="sqas")
            rys = dpool.tile([128, 5, 16], F32, tag="rys", name="rys")
            nt = dpool.tile([128, 5, 16], F32, tag="nt", name="nt")
            nc.vector.tensor_scalar(
                rys.bitcast(mybir.dt.uint32), nh.bitcast(mybir.dt.uint32),
                1, None, op0=mybir.AluOpType.logical_shift_right)
            nc.vector.tensor_tensor(
                rys.bitcast(mybir.dt.uint32),
                magic5[:, :, r0:r1].bitcast(mybir.dt.uint32),
                rys.bitcast(mybir.dt.uint32), op=mybir.AluOpType.subtract)
            nc.vector.tensor_mul(nt, rys, rys)
            nc.vector.tensor_mul(nt, nt, nh)
            nc.vector.tensor_scalar(
                nt, nt, -0.5, 1.5,
                op0=mybir.AluOpType.mult, op1=mybir.AluOpType.add)
            nc.vector.tensor_mul(rys, rys, nt)
            nc.vector.tensor_scalar_add(sqas, nh, 1.0)
            nc.vector.reciprocal(sqas, sqas)
            nc.vector.tensor_mul(sqas, sqas, rys)
            nc.vector.tensor_mul(sqas, nh, sqas)
            for k, (p0, ln) in enumerate(PIX_CHUNKS):
                pm3 = pms[k].rearrange("p (r i) -> p r i", i=D)
                cbf3 = capsbf[:, k].rearrange("p (r i) -> p r i", i=D)
                eng = nc.gpsimd if k < 2 else nc.vector
                eng.tensor_mul(
                    cbf3[:ln, r0:r1], pm3[:ln, r0:r1],
                    sqas[:ln, k].unsqueeze(2).broadcast_to([ln, rn, D]))

        def squash_A(b, craw):
            # pumped og1 squash part 1: fwd transposes + |.|^2 partials.
            # PE ops lead, DVE follows -> no PE stall at the pump slot.
            def f():
                pms = [pmpool.tile([128, 256], F32, tag="pm", name="pm")
                       for _ in PIX_CHUNKS]
                nsqs = dpool.tile([128, 5, R], F32, tag="nsqs", name="nsqs")
                for k, (p0, ln) in enumerate(PIX_CHUNKS):
                    tp = tps.tile([128, 128], F32, tag="tp", name="tp")
                    nc.tensor.transpose(tp[:ln, :], craw[:, 1, p0:p0 + ln],
                                        idf)
                    nc.scalar.activation(pms[k][:ln, 128:256], tp[:ln, :],
                                         AF.Copy)
                for k, (p0, ln) in enumerate(PIX_CHUNKS):
                    pm3 = pms[k].rearrange("p (r i) -> p r i", i=D)
                    sq = dpool.tile([128, 16, D], F32, tag="sqh", name="sqh")
                    eng = nc.gpsimd if k < 2 else nc.vector
                    eng.tensor_mul(sq[:ln], pm3[:ln, 16:32], pm3[:ln, 16:32])
                    nc.vector.reduce_sum(nsqs[:ln, k, 16:32], sq[:ln],
                                         axis=AX.X)
                sqst[b] = [pms, nsqs]
            return f

        def squash_B(b):
            # pumped og1 squash part 2: scale chain + caps scaling (no PE).
            def f():
                pms, nsqs = sqst[b]
                capsbf = capspool.tile([128, 5, 256], BF, tag="cbf",
                                       name="cbf")
                scale_half(pms, nsqs, capsbf, 16, 32)
                sqst[b] = [pms, nsqs, capsbf]
            return f

        def squash_C(b):
            # pumped og1 squash part 3: bwd transposes (deps long done ->
            # PE never waits) + capsT og1 + capsum g1.
            def f():
                pms, nsqs, capsbf = sqst[b]
                capsT = capspool.tile([128, 2, PIX], BF, tag="cT", name="cT")
                capsum = capspool.tile([128, 2], F32, tag="csum",
                                      name="csum")
                for k, (p0, ln) in enumerate(PIX_CHUNKS):
                    tb = tps.tile([128, 128], BF, tag="tp", name="tb")
                    nc.tensor.transpose(
                        tb[:, :ln], capsbf[:ln, k, 128:256], idb[:ln, :ln])
                    nc.vector.tensor_copy(capsT[:, 1, p0:p0 + ln],
                                          tb[:, :ln])
                nc.vector.reduce_sum(capsum[:, 1:2], capsT[:, 1], axis=AX.X)
                sqst[b] = [pms, nsqs, capsbf, capsT, capsum]
            return f

        def squash_finish(b, craw):
            # og0 half at the tail: transposes + partial norms + scale +
            # bwd transposes + capsum g0.
            pms, nsqs, capsbf, capsT, capsum = sqst.pop(b)
            for k, (p0, ln) in enumerate(PIX_CHUNKS):
                tp = tps.tile([128, 128], F32, tag="tp", name="tp")
                nc.tensor.transpose(tp[:ln, :], craw[:, 0, p0:p0 + ln], idf)
                nc.vector.tensor_copy(pms[k][:ln, 0:128], tp[:ln, :])
            for k, (p0, ln) in enumerate(PIX_CHUNKS):
                pm3 = pms[k].rearrange("p (r i) -> p r i", i=D)
                sq = dpool.tile([128, 16, D], F32, tag="sqh", name="sqh")
                eng = nc.gpsimd if k < 2 else nc.vector
                eng.tensor_mul(sq[:ln], pm3[:ln, 0:16], pm3[:ln, 0:16])
                nc.vector.reduce_sum(nsqs[:ln, k, 0:16], sq[:ln],
                                     axis=AX.X)
            scale_half(pms, nsqs, capsbf, 0, 16)
            for k, (p0, ln) in enumerate(PIX_CHUNKS):
                tb = tps.tile([128, 128], BF, tag="tp", name="tb")
                nc.tensor.transpose(
                    tb[:, :ln], capsbf[:ln, k, 0:128], idb[:ln, :ln])
                nc.scalar.activation(capsT[:, 0, p0:p0 + ln], tb[:, :ln],
                                     AF.Copy)
            nc.vector.reduce_sum(capsum[:, 0:1], capsT[:, 0], axis=AX.X)
            return capsbf, capsT, capsum

        # ================= routing (per image, staged) =================
        def register_routing(b, capsbf, capsT, capsum):
            st = {}
            blog = rpool.tile([128, 5, R, C], F32, tag="blog", name="blog")

            def sv_t4(it, last=False):
                # (c,o)-packed layout: partition p = c_local*16+o, halves
                # h=0 (c 0-4) / h=1 (c 5-9).  The per-class s/T4 matmuls
                # batch into 4 + 4 wide ones; the c==c' diagonal is pulled
                # out with a mask multiply + reduce on DVE.
                def f():
                    sF = dpool.tile([80, 2], F32, tag="sF", name="sF")
                    if it == 0:
                        csb = dpool.tile([128, 2], BF, tag="csb", name="csb")
                        nc.vector.tensor_scalar_mul(csb, capsum, 1.0 / C)
                        sps = rps.tile([80, 2], F32, tag="rps", name="sps0")
                        for h in range(2):
                            for m in range(2):
                                nc.tensor.matmul(
                                    sps[:, h:h + 1],
                                    ws_t[m][:, 80 * h:80 * (h + 1)],
                                    csb[:, m:m + 1],
                                    start=(m == 0), stop=(m == 1))
                        nc.vector.tensor_copy(sF, sps)
                    else:
                        Gp = st['Gp']
                        for h in range(2):
                            sps = rps.tile([80, C], F32, tag="rps",
                                           name=f"sps{h}")
                            for m in range(2):
                                nc.tensor.matmul(
                                    sps, ws_t[m][:, 80 * h:80 * (h + 1)],
                                    Gp[m], start=(m == 0), stop=(m == 1))
                            fm = dpool.tile([80, C], F32, tag="sfm",
                                            name="sfm")
                            nc.vector.tensor_mul(fm, sps, m80[:, h])
                            nc.vector.reduce_sum(sF[:, h:h + 1], fm,
                                                 axis=AX.X)
                    # squash on the packed layout: per-class norms via a
                    # block-ones matmul, scale chain on [5, 2], broadcast
                    # back via a K=5 matmul.  All elementwise on DVE.
                    sq2 = dpool.tile([80, 2], F32, tag="sq2", name="sq2")
                    nc.vector.tensor_mul(sq2, sF, sF)
                    n2ps = rps.tile([5, 2], F32, tag="rps", name="n2ps")
                    for h in range(2):
                        nc.tensor.matmul(n2ps[:, h:h + 1], m80[:, 0, :5],
                                         sq2[:, h:h + 1],
                                         start=True, stop=True)
                    n2 = dpool.tile([5, 2], F32, tag="n2", name="n2")
                    nc.vector.tensor_copy(n2, n2ps)
                    ry = dpool.tile([128, 16], F32, tag="ry", name="ry")
                    dve_rsqrt(ry[:5, :2], n2, 5, 2, "vr", iters=1)
                    a2 = dpool.tile([5, 2], F32, tag="a2", name="a2")
                    nc.vector.tensor_scalar_add(a2, n2, 1.0)
                    nc.vector.reciprocal(a2, a2)
                    nc.vector.tensor_mul(a2, a2, ry[:5, :2])
                    nc.vector.tensor_mul(a2, n2, a2)
                    scps = rps.tile([80, 2], F32, tag="rps", name="scps")
                    for h in range(2):
                        nc.tensor.matmul(scps[:, h:h + 1], m580,
                                         a2[:, h:h + 1],
                                         start=True, stop=True)
                    if last:
                        vff = dpool.tile([80, 2], F32, tag="vff", name="vff")
                        nc.vector.tensor_mul(vff, sF, scps)
                        vT = rps.tile([2, 80], F32, tag="rps", name="vT")
                        nc.tensor.transpose(vT, vff, idf[:80, :80])
                        vout = rpool.tile([2, 80], F32, tag="vout",
                                          name="vout")
                        nc.vector.tensor_copy(vout, vT)
                        nc.sync.dma_start(
                            vout_d[b * C:(b + 1) * C, :]
                            .rearrange("(h f) o -> h (f o)", f=5), vout)
                        return
                    vF2 = rpool.tile([80, 2], BF, tag="vF2", name="vF2")
                    nc.vector.tensor_mul(vF2, sF, scps)
                    # T4: rhs = vF broadcast masked to [80, C] per half,
                    # contraction over the packed (c,o) dim in 2 halves.
                    T4 = [rpool.tile([128, R * C], BF, tag=f"T4_{m}",
                                     name=f"T4_{m}") for m in range(2)]
                    vm = dpool.tile([80, 2, C], BF, tag="vm", name="vm")
                    nc.vector.tensor_mul(
                        vm, vF2.unsqueeze(2).broadcast_to([80, 2, C]), m80b)
                    for m in range(2):
                        t4 = rps.tile([128, C], F32, tag="rps", name="t4")
                        for h in range(2):
                            nc.tensor.matmul(
                                t4, wcf[:, h, m * 128:(m + 1) * 128],
                                vm[:, h], start=(h == 0), stop=(h == 1))
                        data = t4.unsqueeze(1).broadcast_to([128, R, C])
                        mk = maskg[m].rearrange("p (r c) -> p r c", c=C)
                        nc.vector.tensor_mul(
                            T4[m].rearrange("p (r c) -> p r c", c=C),
                            data, mk)
                    st['T4'] = T4
                return f

            def dlstage(it):
                def f():
                    T4 = st['T4']
                    for k, (p0, ln) in enumerate(PIX_CHUNKS):
                        dl = rps.tile([128, R, C], F32, tag="rps", name="dl")
                        for kc in range(2):
                            nc.tensor.matmul(
                                dl[:ln], capsT[:, kc, p0:p0 + ln],
                                T4[kc], start=(kc == 0), stop=(kc == 1))
                        if it == 0:
                            nc.vector.tensor_copy(blog[:ln, k], dl[:ln])
                        else:
                            nc.vector.tensor_add(blog[:ln, k], blog[:ln, k],
                                                 dl[:ln])
                return f

            def efstage():
                # exp/softmax-weight + F' matmuls, chunk-pipelined so the
                # F4 accumulation follows each chunk's cw by ~1 op instead
                # of waiting for all five chunks.
                e = rpool.tile([128, 5, R, C], BF, tag="e", name="e")
                cwt = rpool.tile([128, 5, R, D], BF, tag="cw", name="cw")
                F4 = [rps.tile([128, R * C], F32, tag="rps", name=f"F4_{m}")
                      for m in range(2)]
                dens = dpool.tile([128, 5, R], F32, tag="dens", name="dens")
                for k, (p0, ln) in enumerate(PIX_CHUNKS):
                    nc.scalar.activation(e[:ln, k], blog[:ln, k], AF.Exp)
                    nc.vector.reduce_sum(dens[:ln, k], e[:ln, k], axis=AX.X)
                    nc.vector.reciprocal(dens[:ln, k], dens[:ln, k])
                for k, (p0, ln) in enumerate(PIX_CHUNKS):
                    cbf4 = capsbf[:, k].rearrange("p (r i) -> p r i", i=D)
                    eng = nc.gpsimd if k < 2 else nc.vector
                    eng.tensor_mul(
                        cwt[:ln, k], cbf4[:ln],
                        dens[:ln, k].unsqueeze(2).broadcast_to([ln, R, D]))
                    cwf = cwt[:, k].rearrange("p r i -> p (r i)")
                    ef = e[:, k].rearrange("p r c -> p (r c)")
                    for m in range(2):
                        nc.tensor.matmul(F4[m],
                                         cwf[:ln, m * 128:(m + 1) * 128],
                                         ef[:ln],
                                         start=(k == 0), stop=(k == 4))
                Gp = [rpool.tile([128, C], BF, tag=f"G{m}", name=f"G{m}")
                      for m in range(2)]
                for m in range(2):
                    fm = dpool.tile([128, R * C], BF, tag="fm", name="fm")
                    nc.vector.tensor_mul(fm, F4[m], maskg[m])
                    gf = dpool.tile([128, C], F32, tag="gf", name="gf")
                    nc.vector.reduce_sum(
                        gf, fm.rearrange("p (r c) -> p c r", c=C), axis=AX.X)
                    nc.vector.tensor_copy(Gp[m], gf)
                st['Gp'] = Gp

            pending.extend([
                (sv_t4(0), 0), (dlstage(0), 0), (efstage, 0),
                (sv_t4(1), 0), (dlstage(1), 0), (efstage, 0),
                (sv_t4(2, last=True), 0),
            ])

        # ================= main pipeline =================
        h1s = {0: conv1(0)}
        for b in range(B):
            craw = crawpool.tile([128, 2, PIX], F32, tag="craw", name="craw")
            hf = h1cast(h1s[b])
            pending.append((squash_A(b, craw), 300))
            pending.append((squash_B(b), 0))
            pending.append((squash_C(b), 0))
            conv2(b, h1s[b], hf, craw)
            if b + 1 < B:
                h1s[b + 1] = conv1(b + 1)
            drain()  # safety: all pumped stages must be fully emitted
            cbs = squash_finish(b, craw)
            register_routing(b, *cbs)
        drain()  # routing of the last image (exposed tail)

    nc.compile()
    return nc


@functools.lru_cache(maxsize=1)
def _get_nc():
    return _build_nc()


def _prep_consts(conv1_w, conv1_b, conv2_w, conv2_b, route_w):
    bf = ml_dtypes.bfloat16
    f8 = ml_dtypes.float8_e4m3
    f32 = np.float32
    w1 = np.zeros((256, 256), f32)
    w1[:K1] = conv1_w.astype(f32).transpose(1, 2, 3, 0).reshape(K1, 256)
    # DoubleRow lhsT layout [p, j, m] = w1[j*128+p, m], fp8 with x256 scale
    w1dr = np.clip(w1 * 256.0, -240, 240).reshape(2, 128, 256).transpose(1, 0, 2)
    w2 = conv2_w.astype(f32).reshape(2, 128, 2, 128, 81)  # [og, mo, ig, ki, tap]
    # bf16 taps TF8..80: [ig, og, ki, tap, mo], x4096 (exact pow2) so they
    # accumulate at the same scale as the fp8 (x16 * x256) taps
    w2b = (w2[..., TF8:].transpose(2, 0, 3, 4, 1) * 4096.0)
    # fp8 taps 0..TF8-1: [og, ki, tap, ig, mo], x256
    w2f = np.clip(w2[..., :TF8].transpose(0, 3, 4, 2, 1) * 256.0, -240, 240)
    ws = route_w.astype(f32).transpose(0, 2, 1, 3).reshape(256, C * O)
    # wcf[(c_l,o), h, q] = wcob[o, 5h+c_l, q]; wcob[o,c,q] = route_w view
    wcob = route_w.astype(f32).transpose(3, 1, 0, 2).reshape(O, C, 256)
    wcf = np.zeros((80, 2, 256), f32)
    for cl in range(5):
        for o in range(O):
            for h in range(2):
                wcf[cl * 16 + o, h] = wcob[o, 5 * h + cl]
    # m80[p=(c_l,o), h, c'] = (c' == 5h + c_l)
    m80 = np.zeros((80, 2, C), f32)
    for cl in range(5):
        for h in range(2):
            m80[cl * 16:(cl + 1) * 16, h, 5 * h + cl] = 1.0
    # m580[j, p] = (j == p//16)
    m580 = np.zeros((5, 80), f32)
    for j in range(5):
        m580[j, j * 16:(j + 1) * 16] = 1.0
    maskg = np.zeros((2, 128, R * C), f32)
    for m in range(2):
        for j in range(128):
            r = m * 16 + j // D
            maskg[m, j, r * C:(r + 1) * C] = 1.0
    return {
        "w1": np.ascontiguousarray(w1dr).astype(f8),
        "b1": np.ascontiguousarray(conv1_b.astype(f32).reshape(256, 1)),
        "w2b": np.ascontiguousarray(w2b).reshape(2, 2, 128, NTB * 128).astype(bf),
        "w2f": np.ascontiguousarray(w2f).reshape(2, 128, TF8 * 2 * 128).astype(f8),
        "b2": np.ascontiguousarray(conv2_b.astype(f32).reshape(256, 1)),
        "ws": np.ascontiguousarray(ws).astype(bf),
        "wcf": np.ascontiguousarray(wcf).astype(bf),
        "m80": m80,
        "m80b": np.ascontiguousarray(m80).astype(bf),
        "m580": m580,
        "idf": np.eye(128, dtype=f32),
        "idb": np.eye(128, dtype=f32).astype(bf),
        "maskg": maskg,
    }


def _ensure_ntff_hook():
    """The agent image's antenv lacks axon_hooks; shim it so trace=True works."""
    import sys
    import types
    try:
        from antenv import axon_hooks  # noqa: F401
        return
    except ImportError:
        pass
    mod = types.ModuleType("antenv.axon_hooks")
    _h = [None]
    mod.get_axon_ntff_profile_hook = lambda: _h[0]
    mod.set_axon_ntff_profile_hook = lambda h: _h.__setitem__(0, h)
    sys.modules["antenv.axon_hooks"] = mod
    try:
        from trn_agent_boot.trn_boot import _ntff_profile_via_ctypes
        mod.set_axon_ntff_profile_hook(
            _ntff_profile_via_ctypes("/opt/axon/libaxon_pjrt.so"))
    except Exception as e:  # degrade: trace skipped, run still works
        print(f"ntff hook shim failed: {e}")


def run(x, conv1_w, conv1_b, conv2_w, conv2_b, route_w, trace=False, cores=NCORES):
    if trace:
        _ensure_ntff_hook()
    x = np.asarray(x, np.float32)
    nb = x.shape[0]
    consts = _prep_consts(np.asarray(conv1_w), np.asarray(conv1_b),
                          np.asarray(conv2_w), np.asarray(conv2_b),
                          np.asarray(route_w))
    win = np.lib.stride_tricks.sliding_window_view(x, (9, 9), axis=(2, 3))
    xb = (win.transpose(0, 1, 4, 5, 2, 3)          # [b, c, kh, kw, y, x]
          .reshape(nb, K1, NPIX1))
    # fp8 DoubleRow layout [b, p, j, n], rows 243..255 zero, x16 scale
    xq = np.zeros((nb, 256, NPIX1), np.float32)
    xq[:, :K1] = np.clip(xb * 16.0, -240, 240)
    xq = np.ascontiguousarray(
        xq.reshape(nb, 2, 128, NPIX1).transpose(0, 2, 1, 3)
    ).astype(ml_dtypes.float8_e4m3)
    assert nb == B * cores
    in_maps = []
    for cid in range(cores):
        m = dict(consts)
        m["x"] = np.ascontiguousarray(xq[cid * B:(cid + 1) * B])
        in_maps.append(m)
    res = run_bass_kernel_spmd(_get_nc(), in_maps, list(range(cores)), trace=trace)
    out = np.concatenate([r["v_out"].reshape(B, C, O) for r in res.results], axis=0)
    return out.astype(np.float32), res


def kernel(x, conv1_w, conv1_b, conv2_w, conv2_b, route_w):
    out, _ = run(x, conv1_w, conv1_b, conv2_w, conv2_b, route_w, trace=False)
    return out


# revision 13
# speedup vs baseline: 1.4573x; 1.0128x over previous
# CapsuleNetwork Trainium2 kernel (8-core data parallel, 4 images/core).
#
# Per core, fully software-pipelined over images:
#   conv1 3->256 k9 s1 (im2col K=243, fp8 DoubleRow) -> conv2 256->256 k9 s2
#   (81-tap PSUM accumulation, hybrid bf16/fp8) -> squash -> 3-iter routing.
# conv2 runs as 4 (og, y) phases of one PSUM bank each; taps 14..80 run in
# bf16 (w2 host-scaled x4096, exact pow2), taps 0..13 run in fp8 DoubleRow
# (h1 cast x16 on DVE/gpsimd, w2 x256) -- the tap split keeps the final
# rel-err ~0.015 while shaving ~25% of conv2's PE cycles.  Image b's routing
# instructions are interleaved ("pumped") into image b+1's conv2 tap loop so
# the vector/scalar-bound routing hides under the tensor-bound conv2 stream.
# Startup: w1 + the first im2col ride short pixel-sliced pieces on both HWDGE
# queues so conv1(0) starts ~7us earlier; conv1 walks pixel-chunks n-outer to
# consume them in arrival order.  All squash/routing elementwise math runs on
# DVE/gpsimd (single activation table load); softmax/F' stages are fused into
# whole-tile ops to cut the exposed routing tail of the last image.
import functools
from collections import deque
from contextlib import ExitStack

import numpy as np
import ml_dtypes

import concourse.bass as bass
import concourse.tile as tile
from concourse import bacc
from concourse import mybir
from concourse.bass_utils import run_bass_kernel_spmd

BF = mybir.dt.bfloat16
F32 = mybir.dt.float32
AF = mybir.ActivationFunctionType
AX = mybir.AxisListType

NCORES = 8
B = 4              # images per core
K1 = 243           # 3*9*9 im2col contraction
NPIX1 = 3136       # 56*56 conv1 output pixels
N1CH = 448         # conv1 moving chunk (3136 = 7*448)
PIX = 576          # 24*24 conv2 output pixels
PIX_CHUNKS = [(0, 128), (128, 128), (256, 128), (384, 128), (512, 64)]
R, D, C, O = 32, 8, 10, 16
TF8 = 16           # conv2 taps 0..TF8-1 in fp8 DoubleRow, rest bf16
NTB = 81 - TF8     # bf16 taps


def _build_nc():
    nc = bacc.Bacc("TRN2", target_bir_lowering=False, debug=False)
    F8 = mybir.dt.float8e4
    x_d = nc.declare_dram_parameter("x", [B, 128, 2, NPIX1], F8, isOutput=False)
    w1_d = nc.declare_dram_parameter("w1", [128, 2, 256], F8, isOutput=False)
    b1_d = nc.declare_dram_parameter("b1", [256, 1], F32, isOutput=False)
    w2b_d = nc.declare_dram_parameter("w2b", [2, 2, 128, NTB * 128], BF, isOutput=False)
    w2f_d = nc.declare_dram_parameter("w2f", [2, 128, TF8 * 2 * 128], F8, isOutput=False)
    b2_d = nc.declare_dram_parameter("b2", [256, 1], F32, isOutput=False)
    ws_d = nc.declare_dram_parameter("ws", [256, C * O], BF, isOutput=False)
    wcf_d = nc.declare_dram_parameter("wcf", [80, 2, 256], BF, isOutput=False)
    m80_d = nc.declare_dram_parameter("m80", [80, 2, C], F32, isOutput=False)
    m80b_d = nc.declare_dram_parameter("m80b", [80, 2, C], BF, isOutput=False)
    m580_d = nc.declare_dram_parameter("m580", [5, 80], F32, isOutput=False)
    maskg_d = nc.declare_dram_parameter("maskg", [2, 128, R * C], F32, isOutput=False)
    idf_d = nc.declare_dram_parameter("idf", [128, 128], F32, isOutput=False)
    idb_d = nc.declare_dram_parameter("idb", [128, 128], BF, isOutput=False)
    vout_d = nc.declare_dram_parameter("v_out", [B * C, O], F32, isOutput=True)

    with tile.TileContext(nc) as tc, ExitStack() as ctx:
        consts = ctx.enter_context(tc.tile_pool(name="consts", bufs=1))
        w1t = consts.tile([128, 2, 256], mybir.dt.float8e4, tag="w1t",
                          name="w1t")
        b1t = [consts.tile([128, 1], F32, tag=f"b1_{m}", name=f"b1_{m}") for m in range(2)]
        b2t = [consts.tile([128, 1], F32, tag=f"b2_{m}", name=f"b2_{m}") for m in range(2)]
        ws_t = [consts.tile([128, C * O], BF, tag=f"ws{m}", name=f"ws{m}") for m in range(2)]
        wcf = consts.tile([80, 2, 256], BF, tag="wcf", name="wcf")
        m80 = consts.tile([80, 2, C], F32, tag="m80", name="m80")
        m80b = consts.tile([80, 2, C], BF, tag="m80b", name="m80b")
        m580 = consts.tile([5, 80], F32, tag="m580", name="m580")
        idf = consts.tile([128, 128], F32, tag="idf", name="idf")
        idb = consts.tile([128, 128], BF, tag="idb", name="idb")
        maskg = [consts.tile([128, R * C], F32, tag=f"mg{m}", name=f"mg{m}")
                 for m in range(2)]
        # fast-inverse-sqrt magic seed (0x5f3759df) as an f32-bit pattern
        magic = consts.tile([128, 32], F32, tag="magic", name="magic")
        nc.vector.memset(
            magic, float(np.uint32(0x5F3759DF).view(np.float32)))
        magic5 = consts.tile([128, 5, 32], F32, tag="magic5", name="magic5")
        nc.vector.memset(
            magic5, float(np.uint32(0x5F3759DF).view(np.float32)))

        def dve_rsqrt(y, x, p, n, tmp_tag, iters=2):
            """y[:p,:n] = 1/sqrt(x[:p,:n]) on DVE only (bit trick +
            Newton).  No scalar engine -> no act-table thrash."""
            t = dpool.tile([128, 32], F32, tag=f"{tmp_tag}t", name=f"{tmp_tag}t")
            nc.vector.tensor_scalar(
                y.bitcast(mybir.dt.uint32),
                x.bitcast(mybir.dt.uint32), 1, None,
                op0=mybir.AluOpType.logical_shift_right)
            nc.vector.tensor_tensor(
                y.bitcast(mybir.dt.uint32),
                magic[:p, :n].bitcast(mybir.dt.uint32),
                y.bitcast(mybir.dt.uint32),
                op=mybir.AluOpType.subtract)
            for _ in range(iters):  # y *= 1.5 - 0.5*x*y*y
                nc.vector.tensor_mul(t[:p, :n], y, y)
                nc.vector.tensor_mul(t[:p, :n], t[:p, :n], x)
                nc.vector.tensor_scalar(
                    t[:p, :n], t[:p, :n], -0.5, 1.5,
                    op0=mybir.AluOpType.mult, op1=mybir.AluOpType.add)
                nc.vector.tensor_mul(y, y, t[:p, :n])

        # ---- persistent pools (whole-kernel lifetime, ring-buffered) ----
        h1pool = ctx.enter_context(tc.tile_pool(name="h1p", bufs=2))
        h8pool = ctx.enter_context(tc.tile_pool(name="h8p", bufs=2))
        impool = ctx.enter_context(tc.tile_pool(name="imp", bufs=2))
        w2pool = ctx.enter_context(tc.tile_pool(name="w2p", bufs=1))
        crawpool = ctx.enter_context(tc.tile_pool(name="crawp", bufs=2))
        capspool = ctx.enter_context(tc.tile_pool(name="capsp", bufs=2))
        rpool = ctx.enter_context(tc.tile_pool(name="rpool", bufs=2))
        dpool = ctx.enter_context(tc.tile_pool(name="dtmp", bufs=4))
        pmpool = ctx.enter_context(tc.tile_pool(name="pmp", bufs=5))
        cpsum = ctx.enter_context(tc.tile_pool(name="cpsum", bufs=4, space="PSUM"))
        tps = ctx.enter_context(tc.tile_pool(name="tps", bufs=2, space="PSUM"))
        rps = ctx.enter_context(tc.tile_pool(name="rps", bufs=2, space="PSUM"))

        w2bt = [[w2pool.tile([128, NTB, 128], BF, tag=f"w2b_{ig}_{og}",
                             name=f"w2b_{ig}_{og}")
                 for og in range(2)] for ig in range(2)]
        w2ft = [w2pool.tile([128, TF8, 2, 128], mybir.dt.float8e4,
                            tag=f"w2f_{og}", name=f"w2f_{og}")
                for og in range(2)]

        # ================= DMA issue block =================
        # Two HWDGE queues (sync + scalar); scalar's queue stays SHORT (5
        # early issues, no WAR waits) so its relu/exp compute never queues
        # behind DMA issues.  w1 + pixel-sliced im0 pieces lead on both
        # queues so conv1(0) can start ~11us in; w2 og0 pieces follow in
        # tap-consumption order (slice-precise dep tracking unblocks
        # conv2's taps as pieces land).  gpsimd/SWDGE takes the small
        # routing consts.
        for m in range(2):
            nc.gpsimd.dma_start(b1t[m], b1_d[m * 128:(m + 1) * 128, :])
        for m in range(2):
            nc.gpsimd.dma_start(b2t[m], b2_d[m * 128:(m + 1) * 128, :])
        nc.gpsimd.dma_start(idf, idf_d[:, :])
        nc.gpsimd.dma_start(idb, idb_d[:, :])
        for m in range(2):
            nc.gpsimd.dma_start(maskg[m], maskg_d[m])
        for m in range(2):
            nc.gpsimd.dma_start(ws_t[m], ws_d[m * 128:(m + 1) * 128, :])
        nc.gpsimd.dma_start(wcf, wcf_d[:, :, :])
        nc.gpsimd.dma_start(m80, m80_d[:, :, :])
        nc.gpsimd.dma_start(m80b, m80b_d[:, :, :])
        nc.gpsimd.dma_start(m580, m580_d[:, :])

        im = [None] * B

        def issue_im(b):
            imt = impool.tile([128, 2, NPIX1], mybir.dt.float8e4, tag="imA",
                              name="imA")
            nc.sync.dma_start(imt.rearrange("p j n -> p (j n)"),
                              x_d[b].rearrange("p j n -> p (j n)"))
            im[b] = imt

        def w2b_piece(eng, ig, og, t0, t1):
            a, bb = t0 - TF8, t1 - TF8
            eng.dma_start(
                w2bt[ig][og][:, a:bb].rearrange("p t m -> p (t m)"),
                w2b_d[ig, og, :, a * 128:bb * 128])

        PXA, PXS = 448, 1792  # im0 pixel splits (chunk n=0 | n=1-3 | n=4-6)
        im0 = impool.tile([128, 2, NPIX1], mybir.dt.float8e4, tag="imA",
                          name="imA")
        im[0] = im0
        # scalar queue: w1 + im0 front-low + og1 (consumed first) pieces;
        # front im0 pieces split per conv1 chunk so n=0 lands first, and
        # the leading w2b piece is small so conv2's tap stream starts early
        nc.scalar.dma_start(w1t.rearrange("p j n -> p (j n)"),
                            w1_d.rearrange("p j n -> p (j n)"))
        nc.scalar.dma_start(im0[0:64, :, 0:PXA], x_d[0, 0:64, :, 0:PXA])
        nc.scalar.dma_start(im0[0:64, :, PXA:PXS], x_d[0, 0:64, :, PXA:PXS])
        w2b_piece(nc.scalar, 0, 1, TF8, 26)
        w2b_piece(nc.scalar, 0, 1, 26, 46)
        w2b_piece(nc.scalar, 1, 1, 46, 64)
        w2b_piece(nc.scalar, 1, 1, 64, 81)
        w2b_piece(nc.scalar, 0, 0, TF8, 46)
        # sync queue: im0 pieces -> rest of og1 -> fp8 -> og0 -> im1-3
        nc.sync.dma_start(im0[64:128, :, 0:PXA], x_d[0, 64:128, :, 0:PXA])
        nc.sync.dma_start(im0[64:128, :, PXA:PXS], x_d[0, 64:128, :, PXA:PXS])
        w2b_piece(nc.sync, 1, 1, TF8, 26)
        nc.sync.dma_start(im0[0:64, :, PXS:NPIX1], x_d[0, 0:64, :, PXS:NPIX1])
        nc.sync.dma_start(im0[64:128, :, PXS:NPIX1],
                          x_d[0, 64:128, :, PXS:NPIX1])
        w2b_piece(nc.sync, 1, 1, 26, 46)
        w2b_piece(nc.sync, 0, 1, 46, 81)
        nc.sync.dma_start(w2ft[1].rearrange("p t j m -> p (t j m)"), w2f_d[1])
        issue_im(1)
        nc.sync.dma_start(w2ft[0].rearrange("p t j m -> p (t j m)"), w2f_d[0])
        w2b_piece(nc.sync, 1, 0, TF8, 46)
        w2b_piece(nc.sync, 0, 0, 46, 81)
        w2b_piece(nc.sync, 1, 0, 46, 81)
        issue_im(2)
        issue_im(3)

        # ================= stage pump =================
        pending = deque()
        tapctr = [0]
        STAGE_START, STAGE_EVERY = 16, 56

        def pump():
            tapctr[0] += 1
            if (pending and tapctr[0] >= STAGE_START
                    and (tapctr[0] - STAGE_START) % STAGE_EVERY == 0
                    and tapctr[0] >= pending[0][1]):
                pending.popleft()[0]()

        def drain():
            while pending:
                pending.popleft()[0]()

        # ================= per-image phases =================
        def conv1(b):
            # fp8 DoubleRow: K=256 (two 128-row k-groups) per instruction;
            # host pre-scales x by 16 and w1 by 256, undone by the relu
            # drain's 2^-12 activation scale.  n-outer so pixel chunks are
            # consumed in DMA arrival order.
            imt = im[b]
            h1t = h1pool.tile([128, 2, 56, 2, 28], BF, tag="h1t", name="h1t")
            for n in range(7):
                for m in range(2):
                    ps = cpsum.tile([128, 8, 56], F32, tag="cps", name="c1ps")
                    nc.tensor.matmul(ps, w1t[:, :, m * 128:(m + 1) * 128],
                                     imt[:, :, n * N1CH:(n + 1) * N1CH],
                                     start=True, stop=True,
                                     perf_mode=mybir.MatmulPerfMode.DoubleRow)
                    # single drain per (n, m): phase-interleave via strided
                    # view; alternate ACT/DVE so drains never pace conv1.
                    ps_v = ps.rearrange("p r (x2 ph) -> p r ph x2", ph=2)
                    dst = h1t[:, m, 8 * n:8 * n + 8, :, :]
                    if n % 2 == 0:
                        nc.scalar.activation(dst, ps_v, AF.Relu,
                                             bias=b1t[m], scale=2.0 ** -12)
                    else:
                        tmp = dpool.tile([128, 8, 2, 28], F32, tag="c1t",
                                         name="c1t")
                        nc.vector.tensor_scalar(
                            tmp, ps_v, 2.0 ** -12, b1t[m],
                            op0=mybir.AluOpType.mult,
                            op1=mybir.AluOpType.add)
                        nc.vector.tensor_scalar_max(dst, tmp, 0.0)
            return h1t

        def h1cast(h1t):
            # fp8 copy of h1 (x16) for the DoubleRow taps; one DVE op
            # (~3.6us), hidden under conv2's leading bf16 taps.  gpsimd
            # takes ~14ns/elem for fp8 stores -- keep it away from this.
            hf = h8pool.tile([128, 2, 56, 2, 28], mybir.dt.float8e4,
                             tag="h1f8", name="h1f8")
            nc.vector.tensor_scalar_mul(hf, h1t, 16.0)
            return hf

        def conv2(b, h1t, h1f8, craw):
            # 4 single-bank phases (og, y), og1 FIRST so og1's full squash
            # half can pump into og0's tap windows.  bf16 taps first (w2
            # x4096), fp8 DoubleRow taps last (so the h1 cast hides under
            # bf16); both accumulate at the same 2^12 scale, undone in the
            # drain.
            tapctr[0] = 0
            for og in (1, 0):
                for y in range(2):
                    ps = cpsum.tile([128, 288], F32, tag="cps", name="c2ps")
                    for t81 in range(TF8, 81):
                        kh, kw = t81 // 9, t81 % 9
                        for ig in range(2):
                            rhs = h1t[:, ig,
                                      kh + 24 * y:kh + 24 * y + 24:2,
                                      kw % 2, kw // 2:kw // 2 + 24]
                            nc.tensor.matmul(
                                ps, w2bt[ig][og][:, t81 - TF8, :], rhs,
                                start=(t81 == TF8 and ig == 0), stop=False)
                            pump()
                    for t81 in range(TF8):
                        kh, kw = t81 // 9, t81 % 9
                        rhs = h1f8[:, :,
                                   kh + 24 * y:kh + 24 * y + 24:2,
                                   kw % 2, kw // 2:kw // 2 + 24]
                        nc.tensor.matmul(
                            ps, w2ft[og][:, t81], rhs,
                            start=False, stop=(t81 == TF8 - 1),
                            perf_mode=mybir.MatmulPerfMode.DoubleRow)
                        pump()
                    # drain on DVE (keeps scalar's act table on Exp)
                    nc.vector.tensor_scalar(
                        craw[:, og, y * 288:(y + 1) * 288], ps,
                        2.0 ** -12, b2t[og],
                        op0=mybir.AluOpType.mult, op1=mybir.AluOpType.add)

        sqst = {}

        def scale_half(pms, nsqs, capsbf, r0, r1):
            # scale = n * rsqrt(n) / (1+n) for routes r0:r1, all on DVE,
            # chunk-packed; then scale the caps into capsbf (DVE/gpsimd).
            rn = r1 - r0
            nh = nsqs[:, :, r0:r1]
            sqas = dpool.tile([128, 5, 16], F32, tag="sqas", name="sqas")
            rys = dpool.tile([128, 5, 16], F32, tag="rys", name="rys")
            nt = dpool.tile([128, 5, 16], F32, tag="nt", name="nt")
            nc.vector.tensor_scalar(
                rys.bitcast(mybir.dt.uint32), nh.bitcast(mybir.dt.uint32),
                1, None, op0=mybir.AluOpType.logical_shift_right)
            nc.vector.tensor_tensor(
                rys.bitcast(mybir.dt.uint32),
                magic5[:, :, r0:r1].bitcast(mybir.dt.uint32),
                rys.bitcast(mybir.dt.uint32), op=mybir.AluOpType.subtract)
            nc.vector.tensor_mul(nt, rys, rys)
            nc.vector.tensor_mul(nt, nt, nh)
            nc.vector.tensor_scalar(
                nt, nt, -0.5, 1.5,
                op0=mybir.AluOpType.mult, op1=mybir.AluOpType.add)
            nc.vector.tensor_mul(rys, rys, nt)
            nc.vector.tensor_scalar_add(sqas, nh, 1.0)
            nc.vector.reciprocal(sqas, sqas)
            nc.vector.tensor_mul(sqas, sqas, rys)
            nc.vector.tensor_mul(sqas, nh, sqas)
            for k, (p0, ln) in enumerate(PIX_CHUNKS):
                pm3 = pms[k].rearrange("p (r i) -> p r i", i=D)
                cbf3 = capsbf[:, k].rearrange("p (r i) -> p r i", i=D)
                eng = nc.gpsimd if k < 2 else nc.vector
                eng.tensor_mul(
                    cbf3[:ln, r0:r1], pm3[:ln, r0:r1],
                    sqas[:ln, k].unsqueeze(2).broadcast_to([ln, rn, D]))

        def squash_A(b, craw):
            # pumped og1 squash part 1: fwd transposes + |.|^2 partials.
            # PE ops lead, DVE follows -> no PE stall at the pump slot.
            def f():
                pms = [pmpool.tile([128, 256], F32, tag="pm", name="pm")
                       for _ in PIX_CHUNKS]
                nsqs = dpool.tile([128, 5, R], F32, tag="nsqs", name="nsqs")
                for k, (p0, ln) in enumerate(PIX_CHUNKS):
                    tp = tps.tile([128, 128], BF, tag="tp", name="tp")
                    nc.tensor.transpose(tp[:ln, :], craw[:, 1, p0:p0 + ln],
                                        idb)
                    nc.scalar.activation(pms[k][:ln, 128:256], tp[:ln, :],
                                         AF.Copy)
                for k, (p0, ln) in enumerate(PIX_CHUNKS):
                    pm3 = pms[k].rearrange("p (r i) -> p r i", i=D)
                    sq = dpool.tile([128, 16, D], F32, tag="sqh", name="sqh")
                    eng = nc.gpsimd if k < 2 else nc.vector
                    eng.tensor_mul(sq[:ln], pm3[:ln, 16:32], pm3[:ln, 16:32])
                    nc.vector.reduce_sum(nsqs[:ln, k, 16:32], sq[:ln],
                                         axis=AX.X)
                sqst[b] = [pms, nsqs]
            return f

        def squash_B(b):
            # pumped og1 squash part 2: scale chain + caps scaling (no PE).
            def f():
                pms, nsqs = sqst[b]
                capsbf = capspool.tile([128, 5, 256], BF, tag="cbf",
                                       name="cbf")
                scale_half(pms, nsqs, capsbf, 16, 32)
                sqst[b] = [pms, nsqs, capsbf]
            return f

        def squash_C(b):
            # pumped og1 squash part 3: bwd transposes (deps long done ->
            # PE never waits) + capsT og1 + capsum g1.
            def f():
                pms, nsqs, capsbf = sqst[b]
                capsT = capspool.tile([128, 2, PIX], BF, tag="cT", name="cT")
                capsum = capspool.tile([128, 2], F32, tag="csum",
                                      name="csum")
                for k, (p0, ln) in enumerate(PIX_CHUNKS):
                    tb = tps.tile([128, 128], BF, tag="tp", name="tb")
                    nc.tensor.transpose(
                        tb[:, :ln], capsbf[:ln, k, 128:256], idb[:ln, :ln])
                    nc.vector.tensor_copy(capsT[:, 1, p0:p0 + ln],
                                          tb[:, :ln])
                nc.vector.reduce_sum(capsum[:, 1:2], capsT[:, 1], axis=AX.X)
                sqst[b] = [pms, nsqs, capsbf, capsT, capsum]
            return f

        def squash_finish(b, craw):
            # og0 half at the tail: transposes + partial norms + scale +
            # bwd transposes + capsum g0.
            pms, nsqs, capsbf, capsT, capsum = sqst.pop(b)
            for k, (p0, ln) in enumerate(PIX_CHUNKS):
                tp = tps.tile([128, 128], BF, tag="tp", name="tp")
                nc.tensor.transpose(tp[:ln, :], craw[:, 0, p0:p0 + ln], idb)
                nc.vector.tensor_copy(pms[k][:ln, 0:128], tp[:ln, :])
            for k, (p0, ln) in enumerate(PIX_CHUNKS):
                pm3 = pms[k].rearrange("p (r i) -> p r i", i=D)
                sq = dpool.tile([128, 16, D], F32, tag="sqh", name="sqh")
                eng = nc.gpsimd if k < 2 else nc.vector
                eng.tensor_mul(sq[:ln], pm3[:ln, 0:16], pm3[:ln, 0:16])
                nc.vector.reduce_sum(nsqs[:ln, k, 0:16], sq[:ln],
                                     axis=AX.X)
            scale_half(pms, nsqs, capsbf, 0, 16)
            for k, (p0, ln) in enumerate(PIX_CHUNKS):
                tb = tps.tile([128, 128], BF, tag="tp", name="tb")
                nc.tensor.transpose(
                    tb[:, :ln], capsbf[:ln, k, 0:128], idb[:ln, :ln])
                nc.scalar.activation(capsT[:, 0, p0:p0 + ln], tb[:, :ln],
                                     AF.Copy)
            nc.vector.reduce_sum(capsum[:, 0:1], capsT[:, 0], axis=AX.X)
            return capsbf, capsT, capsum

        # ================= routing (per image, staged) =================
        def register_routing(b, capsbf, capsT, capsum):
            st = {}
            blog = rpool.tile([128, 5, R, C], F32, tag="blog", name="blog")

            def sv_t4(it, last=False):
                # (c,o)-packed layout: partition p = c_local*16+o, halves
                # h=0 (c 0-4) / h=1 (c 5-9).  The per-class s/T4 matmuls
                # batch into 4 + 4 wide ones; the c==c' diagonal is pulled
                # out with a mask multiply + reduce on DVE.
                def f():
                    sF = dpool.tile([80, 2], F32, tag="sF", name="sF")
                    if it == 0:
                        csb = dpool.tile([128, 2], BF, tag="csb", name="csb")
                        nc.vector.tensor_scalar_mul(csb, capsum, 1.0 / C)
                        sps = rps.tile([80, 2], F32, tag="rps", name="sps0")
                        for h in range(2):
                            for m in range(2):
                                nc.tensor.matmul(
                                    sps[:, h:h + 1],
                                    ws_t[m][:, 80 * h:80 * (h + 1)],
                                    csb[:, m:m + 1],
                                    start=(m == 0), stop=(m == 1))
                        nc.vector.tensor_copy(sF, sps)
                    else:
                        Gp = st['Gp']
                        for h in range(2):
                            sps = rps.tile([80, C], F32, tag="rps",
                                           name=f"sps{h}")
                            for m in range(2):
                                nc.tensor.matmul(
                                    sps, ws_t[m][:, 80 * h:80 * (h + 1)],
                                    Gp[m], start=(m == 0), stop=(m == 1))
                            fm = dpool.tile([80, C], F32, tag="sfm",
                                            name="sfm")
                            nc.vector.tensor_mul(fm, sps, m80[:, h])
                            nc.vector.reduce_sum(sF[:, h:h + 1], fm,
                                                 axis=AX.X)
                    # squash on the packed layout: per-class norms via a
                    # block-ones matmul, scale chain on [5, 2], broadcast
                    # back via a K=5 matmul.  All elementwise on DVE.
                    sq2 = dpool.tile([80, 2], F32, tag="sq2", name="sq2")
                    nc.vector.tensor_mul(sq2, sF, sF)
                    n2ps = rps.tile([5, 2], F32, tag="rps", name="n2ps")
                    for h in range(2):
                        nc.tensor.matmul(n2ps[:, h:h + 1], m80[:, 0, :5],
                                         sq2[:, h:h + 1],
                                         start=True, stop=True)
                    n2 = dpool.tile([5, 2], F32, tag="n2", name="n2")
                    nc.vector.tensor_copy(n2, n2ps)
                    ry = dpool.tile([128, 16], F32, tag="ry", name="ry")
                    dve_rsqrt(ry[:5, :2], n2, 5, 2, "vr", iters=1)
                    a2 = dpool.tile([5, 2], F32, tag="a2", name="a2")
                    nc.vector.tensor_scalar_add(a2, n2, 1.0)
                    nc.vector.reciprocal(a2, a2)
                    nc.vector.tensor_mul(a2, a2, ry[:5, :2])
                    nc.vector.tensor_mul(a2, n2, a2)
                    scps = rps.tile([80, 2], F32, tag="rps", name="scps")
                    for h in range(2):
                        nc.tensor.matmul(scps[:, h:h + 1], m580,
                                         a2[:, h:h + 1],
                                         start=True, stop=True)
                    if last:
                        vff = dpool.tile([80, 2], F32, tag="vff", name="vff")
                        nc.vector.tensor_mul(vff, sF, scps)
                        vT = rps.tile([2, 80], F32, tag="rps", name="vT")
                        nc.tensor.transpose(vT, vff, idf[:80, :80])
                        vout = rpool.tile([2, 80], F32, tag="vout",
                                          name="vout")
                        nc.vector.tensor_copy(vout, vT)
                        nc.sync.dma_start(
                            vout_d[b * C:(b + 1) * C, :]
                            .rearrange("(h f) o -> h (f o)", f=5), vout)
                        return
                    vF2 = rpool.tile([80, 2], BF, tag="vF2", name="vF2")
                    nc.vector.tensor_mul(vF2, sF, scps)
                    # T4: rhs = vF broadcast masked to [80, C] per half,
                    # contraction over the packed (c,o) dim in 2 halves.
                    T4 = [rpool.tile([128, R * C], BF, tag=f"T4_{m}",
                                     name=f"T4_{m}") for m in range(2)]
                    vm = dpool.tile([80, 2, C], BF, tag="vm", name="vm")
                    nc.vector.tensor_mul(
                        vm, vF2.unsqueeze(2).broadcast_to([80, 2, C]), m80b)
                    for m in range(2):
                        t4 = rps.tile([128, C], F32, tag="rps", name="t4")
                        for h in range(2):
                            nc.tensor.matmul(
                                t4, wcf[:, h, m * 128:(m + 1) * 128],
                                vm[:, h], start=(h == 0), stop=(h == 1))
                        data = t4.unsqueeze(1).broadcast_to([128, R, C])
                        mk = maskg[m].rearrange("p (r c) -> p r c", c=C)
                        nc.vector.tensor_mul(
                            T4[m].rearrange("p (r c) -> p r c", c=C),
                            data, mk)
                    st['T4'] = T4
                return f

            def dlstage(it):
                def f():
                    T4 = st['T4']
                    for k, (p0, ln) in enumerate(PIX_CHUNKS):
                        dl = rps.tile([128, R, C], F32, tag="rps", name="dl")
                        for kc in range(2):
                            nc.tensor.matmul(
                                dl[:ln], capsT[:, kc, p0:p0 + ln],
                                T4[kc], start=(kc == 0), stop=(kc == 1))
                        if it == 0:
                            nc.vector.tensor_copy(blog[:ln, k], dl[:ln])
                        else:
                            nc.vector.tensor_add(blog[:ln, k], blog[:ln, k],
                                                 dl[:ln])
                return f

            def efstage():
                # exp/softmax-weight + F' matmuls, chunk-pipelined so the
                # F4 accumulation follows each chunk's cw by ~1 op instead
                # of waiting for all five chunks.
                e = rpool.tile([128, 5, R, C], BF, tag="e", name="e")
                cwt = rpool.tile([128, 5, R, D], BF, tag="cw", name="cw")
                F4 = [rps.tile([128, R * C], F32, tag="rps", name=f"F4_{m}")
                      for m in range(2)]
                dens = dpool.tile([128, 5, R], F32, tag="dens", name="dens")
                for k, (p0, ln) in enumerate(PIX_CHUNKS):
                    nc.scalar.activation(e[:ln, k], blog[:ln, k], AF.Exp)
                    nc.vector.reduce_sum(dens[:ln, k], e[:ln, k], axis=AX.X)
                    nc.vector.reciprocal(dens[:ln, k], dens[:ln, k])
                for k, (p0, ln) in enumerate(PIX_CHUNKS):
                    cbf4 = capsbf[:, k].rearrange("p (r i) -> p r i", i=D)
                    eng = nc.gpsimd if k < 2 else nc.vector
                    eng.tensor_mul(
                        cwt[:ln, k], cbf4[:ln],
                        dens[:ln, k].unsqueeze(2).broadcast_to([ln, R, D]))
                    cwf = cwt[:, k].rearrange("p r i -> p (r i)")
                    ef = e[:, k].rearrange("p r c -> p (r c)")
                    for m in range(2):
                        nc.tensor.matmul(F4[m],
                                         cwf[:ln, m * 128:(m + 1) * 128],
                                         ef[:ln],
                                         start=(k == 0), stop=(k == 4))
                Gp = [rpool.tile([128, C], BF, tag=f"G{m}", name=f"G{m}")
                      for m in range(2)]
                for m in range(2):
                    fm = dpool.tile([128, R * C], BF, tag="fm", name="fm")
                    nc.vector.tensor_mul(fm, F4[m], maskg[m])
                    gf = dpool.tile([128, C], F32, tag="gf", name="gf")
                    nc.vector.reduce_sum(
                        gf, fm.rearrange("p (r c) -> p c r", c=C), axis=AX.X)
                    nc.vector.tensor_copy(Gp[m], gf)
                st['Gp'] = Gp

            pending.extend([
                (sv_t4(0), 0), (dlstage(0), 0), (efstage, 0),
                (sv_t4(1), 0), (dlstage(1), 0), (efstage, 0),
                (sv_t4(2, last=True), 0),
            ])

        # ================= main pipeline =================
        h1s = {0: conv1(0)}
        for b in range(B):
            craw = crawpool.tile([128, 2, PIX], BF, tag="craw", name="craw")
            hf = h1cast(h1s[b])
            pending.append((squash_A(b, craw), 310))
            pending.append((squash_B(b), 0))
            pending.append((squash_C(b), 0))
            conv2(b, h1s[b], hf, craw)
            if b + 1 < B:
                h1s[b + 1] = conv1(b + 1)
            drain()  # safety: all pumped stages must be fully emitted
            cbs = squash_finish(b, craw)
            register_routing(b, *cbs)
        drain()  # routing of the last image (exposed tail)

    nc.compile()
    return nc


@functools.lru_cache(maxsize=1)
def _get_nc():
    return _build_nc()


def _prep_consts(conv1_w, conv1_b, conv2_w, conv2_b, route_w):
    bf = ml_dtypes.bfloat16
    f8 = ml_dtypes.float8_e4m3
    f32 = np.float32
    w1 = np.zeros((256, 256), f32)
    w1[:K1] = conv1_w.astype(f32).transpose(1, 2, 3, 0).reshape(K1, 256)
    # DoubleRow lhsT layout [p, j, m] = w1[j*128+p, m], fp8 with x256 scale
    w1dr = np.clip(w1 * 256.0, -240, 240).reshape(2, 128, 256).transpose(1, 0, 2)
    w2 = conv2_w.astype(f32).reshape(2, 128, 2, 128, 81)  # [og, mo, ig, ki, tap]
    # bf16 taps TF8..80: [ig, og, ki, tap, mo], x4096 (exact pow2) so they
    # accumulate at the same scale as the fp8 (x16 * x256) taps
    w2b = (w2[..., TF8:].transpose(2, 0, 3, 4, 1) * 4096.0)
    # fp8 taps 0..TF8-1: [og, ki, tap, ig, mo], x256
    w2f = np.clip(w2[..., :TF8].transpose(0, 3, 4, 2, 1) * 256.0, -240, 240)
    ws = route_w.astype(f32).transpose(0, 2, 1, 3).reshape(256, C * O)
    # wcf[(c_l,o), h, q] = wcob[o, 5h+c_l, q]; wcob[o,c,q] = route_w view
    wcob = route_w.astype(f32).transpose(3, 1, 0, 2).reshape(O, C, 256)
    wcf = np.zeros((80, 2, 256), f32)
    for cl in range(5):
        for o in range(O):
            for h in range(2):
                wcf[cl * 16 + o, h] = wcob[o, 5 * h + cl]
    # m80[p=(c_l,o), h, c'] = (c' == 5h + c_l)
    m80 = np.zeros((80, 2, C), f32)
    for cl in range(5):
        for h in range(2):
            m80[cl * 16:(cl + 1) * 16, h, 5 * h + cl] = 1.0
    # m580[j, p] = (j == p//16)
    m580 = np.zeros((5, 80), f32)
    for j in range(5):
        m580[j, j * 16:(j + 1) * 16] = 1.0
    maskg = np.zeros((2, 128, R * C), f32)
    for m in range(2):
        for j in range(128):
            r = m * 16 + j // D
            maskg[m, j, r * C:(r + 1) * C] = 1.0
    return {
        "w1": np.ascontiguousarray(w1dr).astype(f8),
        "b1": np.ascontiguousarray(conv1_b.astype(f32).reshape(256, 1)),
        "w2b": np.ascontiguousarray(w2b).reshape(2, 2, 128, NTB * 128).astype(bf),
        "w2f": np.ascontiguousarray(w2f).reshape(2, 128, TF8 * 2 * 128).astype(f8),
        "b2": np.ascontiguousarray(conv2_b.astype(f32).reshape(256, 1)),
        "ws": np.ascontiguousarray(ws).astype(bf),
        "wcf": np.ascontiguousarray(wcf).astype(bf),
        "m80": m80,
        "m80b": np.ascontiguousarray(m80).astype(bf),
        "m580": m580,
        "idf": np.eye(128, dtype=f32),
        "idb": np.eye(128, dtype=f32).astype(bf),
        "maskg": maskg,
    }


def _ensure_ntff_hook():
    """The agent image's antenv lacks axon_hooks; shim it so trace=True works."""
    import sys
    import types
    try:
        from antenv import axon_hooks  # noqa: F401
        return
    except ImportError:
        pass
    mod = types.ModuleType("antenv.axon_hooks")
    _h = [None]
    mod.get_axon_ntff_profile_hook = lambda: _h[0]
    mod.set_axon_ntff_profile_hook = lambda h: _h.__setitem__(0, h)
    sys.modules["antenv.axon_hooks"] = mod
    try:
        from trn_agent_boot.trn_boot import _ntff_profile_via_ctypes
        mod.set_axon_ntff_profile_hook(
            _ntff_profile_via_ctypes("/opt/axon/libaxon_pjrt.so"))
    except Exception as e:  # degrade: trace skipped, run still works
        print(f"ntff hook shim failed: {e}")


def run(x, conv1_w, conv1_b, conv2_w, conv2_b, route_w, trace=False, cores=NCORES):
    if trace:
        _ensure_ntff_hook()
    x = np.asarray(x, np.float32)
    nb = x.shape[0]
    consts = _prep_consts(np.asarray(conv1_w), np.asarray(conv1_b),
                          np.asarray(conv2_w), np.asarray(conv2_b),
                          np.asarray(route_w))
    win = np.lib.stride_tricks.sliding_window_view(x, (9, 9), axis=(2, 3))
    xb = (win.transpose(0, 1, 4, 5, 2, 3)          # [b, c, kh, kw, y, x]
          .reshape(nb, K1, NPIX1))
    # fp8 DoubleRow layout [b, p, j, n], rows 243..255 zero, x16 scale
    xq = np.zeros((nb, 256, NPIX1), np.float32)
    xq[:, :K1] = np.clip(xb * 16.0, -240, 240)
    xq = np.ascontiguousarray(
        xq.reshape(nb, 2, 128, NPIX1).transpose(0, 2, 1, 3)
    ).astype(ml_dtypes.float8_e4m3)
    assert nb == B * cores
    in_maps = []
    for cid in range(cores):
        m = dict(consts)
        m["x"] = np.ascontiguousarray(xq[cid * B:(cid + 1) * B])
        in_maps.append(m)
    res = run_bass_kernel_spmd(_get_nc(), in_maps, list(range(cores)), trace=trace)
    out = np.concatenate([r["v_out"].reshape(B, C, O) for r in res.results], axis=0)
    return out.astype(np.float32), res


def kernel(x, conv1_w, conv1_b, conv2_w, conv2_b, route_w):
    out, _ = run(x, conv1_w, conv1_b, conv2_w, conv2_b, route_w, trace=False)
    return out


# revision 14
# speedup vs baseline: 1.4851x; 1.0191x over previous
# CapsuleNetwork Trainium2 kernel (8-core data parallel, 4 images/core).
#
# Per core, fully software-pipelined over images:
#   conv1 3->256 k9 s1 (im2col K=243, fp8 DoubleRow) -> conv2 256->256 k9 s2
#   (81-tap PSUM accumulation, hybrid bf16/fp8) -> squash -> 3-iter routing.
# conv2 runs as 4 (og, y) phases of one PSUM bank each; taps 14..80 run in
# bf16 (w2 host-scaled x4096, exact pow2), taps 0..13 run in fp8 DoubleRow
# (h1 cast x16 on DVE/gpsimd, w2 x256) -- the tap split keeps the final
# rel-err ~0.015 while shaving ~25% of conv2's PE cycles.  Image b's routing
# instructions are interleaved ("pumped") into image b+1's conv2 tap loop so
# the vector/scalar-bound routing hides under the tensor-bound conv2 stream.
# Startup: w1 + the first im2col ride short pixel-sliced pieces on both HWDGE
# queues so conv1(0) starts ~7us earlier; conv1 walks pixel-chunks n-outer to
# consume them in arrival order.  All squash/routing elementwise math runs on
# DVE/gpsimd (single activation table load); softmax/F' stages are fused into
# whole-tile ops to cut the exposed routing tail of the last image.
import functools
from collections import deque
from contextlib import ExitStack

import numpy as np
import ml_dtypes

import concourse.bass as bass
import concourse.tile as tile
from concourse import bacc
from concourse import mybir
from concourse.bass_utils import run_bass_kernel_spmd

BF = mybir.dt.bfloat16
F32 = mybir.dt.float32
AF = mybir.ActivationFunctionType
AX = mybir.AxisListType

NCORES = 8
B = 4              # images per core
K1 = 243           # 3*9*9 im2col contraction
NPIX1 = 3136       # 56*56 conv1 output pixels
N1CH = 448         # conv1 moving chunk (3136 = 7*448)
PIX = 576          # 24*24 conv2 output pixels
PIX_CHUNKS = [(0, 128), (128, 128), (256, 128), (384, 128), (512, 64)]
R, D, C, O = 32, 8, 10, 16
TF8 = 18           # conv2 taps 0..TF8-1 in fp8 DoubleRow, rest bf16
NTB = 81 - TF8     # bf16 taps


def _build_nc():
    nc = bacc.Bacc("TRN2", target_bir_lowering=False, debug=False)
    F8 = mybir.dt.float8e4
    x_d = nc.declare_dram_parameter("x", [B, 128, 2, NPIX1], F8, isOutput=False)
    w1_d = nc.declare_dram_parameter("w1", [128, 2, 256], F8, isOutput=False)
    b1_d = nc.declare_dram_parameter("b1", [256, 1], F32, isOutput=False)
    w2b_d = nc.declare_dram_parameter("w2b", [2, 2, 128, NTB * 128], BF, isOutput=False)
    w2f_d = nc.declare_dram_parameter("w2f", [2, 128, TF8 * 2 * 128], F8, isOutput=False)
    b2_d = nc.declare_dram_parameter("b2", [256, 1], F32, isOutput=False)
    ws_d = nc.declare_dram_parameter("ws", [256, C * O], BF, isOutput=False)
    wcf_d = nc.declare_dram_parameter("wcf", [80, 2, 256], BF, isOutput=False)
    m80_d = nc.declare_dram_parameter("m80", [80, 2, C], F32, isOutput=False)
    m80b_d = nc.declare_dram_parameter("m80b", [80, 2, C], BF, isOutput=False)
    m580_d = nc.declare_dram_parameter("m580", [5, 80], F32, isOutput=False)
    maskg_d = nc.declare_dram_parameter("maskg", [2, 128, R * C], F32, isOutput=False)
    idf_d = nc.declare_dram_parameter("idf", [128, 128], F32, isOutput=False)
    idb_d = nc.declare_dram_parameter("idb", [128, 128], BF, isOutput=False)
    vout_d = nc.declare_dram_parameter("v_out", [B * C, O], F32, isOutput=True)

    with tile.TileContext(nc) as tc, ExitStack() as ctx:
        consts = ctx.enter_context(tc.tile_pool(name="consts", bufs=1))
        w1t = consts.tile([128, 2, 256], mybir.dt.float8e4, tag="w1t",
                          name="w1t")
        b1t = [consts.tile([128, 1], F32, tag=f"b1_{m}", name=f"b1_{m}") for m in range(2)]
        b2t = [consts.tile([128, 1], F32, tag=f"b2_{m}", name=f"b2_{m}") for m in range(2)]
        ws_t = [consts.tile([128, C * O], BF, tag=f"ws{m}", name=f"ws{m}") for m in range(2)]
        wcf = consts.tile([80, 2, 256], BF, tag="wcf", name="wcf")
        m80 = consts.tile([80, 2, C], F32, tag="m80", name="m80")
        m80b = consts.tile([80, 2, C], BF, tag="m80b", name="m80b")
        m580 = consts.tile([5, 80], F32, tag="m580", name="m580")
        idf = consts.tile([128, 128], F32, tag="idf", name="idf")
        idb = consts.tile([128, 128], BF, tag="idb", name="idb")
        maskg = [consts.tile([128, R * C], F32, tag=f"mg{m}", name=f"mg{m}")
                 for m in range(2)]
        # fast-inverse-sqrt magic seed (0x5f3759df) as an f32-bit pattern
        magic = consts.tile([128, 32], F32, tag="magic", name="magic")
        nc.vector.memset(
            magic, float(np.uint32(0x5F3759DF).view(np.float32)))
        magic5 = consts.tile([128, 5, 32], F32, tag="magic5", name="magic5")
        nc.vector.memset(
            magic5, float(np.uint32(0x5F3759DF).view(np.float32)))

        def dve_rsqrt(y, x, p, n, tmp_tag, iters=2):
            """y[:p,:n] = 1/sqrt(x[:p,:n]) on DVE only (bit trick +
            Newton).  No scalar engine -> no act-table thrash."""
            t = dpool.tile([128, 32], F32, tag=f"{tmp_tag}t", name=f"{tmp_tag}t")
            nc.vector.tensor_scalar(
                y.bitcast(mybir.dt.uint32),
                x.bitcast(mybir.dt.uint32), 1, None,
                op0=mybir.AluOpType.logical_shift_right)
            nc.vector.tensor_tensor(
                y.bitcast(mybir.dt.uint32),
                magic[:p, :n].bitcast(mybir.dt.uint32),
                y.bitcast(mybir.dt.uint32),
                op=mybir.AluOpType.subtract)
            for _ in range(iters):  # y *= 1.5 - 0.5*x*y*y
                nc.vector.tensor_mul(t[:p, :n], y, y)
                nc.vector.tensor_mul(t[:p, :n], t[:p, :n], x)
                nc.vector.tensor_scalar(
                    t[:p, :n], t[:p, :n], -0.5, 1.5,
                    op0=mybir.AluOpType.mult, op1=mybir.AluOpType.add)
                nc.vector.tensor_mul(y, y, t[:p, :n])

        # ---- persistent pools (whole-kernel lifetime, ring-buffered) ----
        h1pool = ctx.enter_context(tc.tile_pool(name="h1p", bufs=2))
        h8pool = ctx.enter_context(tc.tile_pool(name="h8p", bufs=2))
        impool = ctx.enter_context(tc.tile_pool(name="imp", bufs=2))
        w2pool = ctx.enter_context(tc.tile_pool(name="w2p", bufs=1))
        crawpool = ctx.enter_context(tc.tile_pool(name="crawp", bufs=2))
        capspool = ctx.enter_context(tc.tile_pool(name="capsp", bufs=2))
        rpool = ctx.enter_context(tc.tile_pool(name="rpool", bufs=2))
        dpool = ctx.enter_context(tc.tile_pool(name="dtmp", bufs=4))
        pmpool = ctx.enter_context(tc.tile_pool(name="pmp", bufs=5))
        cpsum = ctx.enter_context(tc.tile_pool(name="cpsum", bufs=4, space="PSUM"))
        tps = ctx.enter_context(tc.tile_pool(name="tps", bufs=2, space="PSUM"))
        rps = ctx.enter_context(tc.tile_pool(name="rps", bufs=2, space="PSUM"))

        w2bt = [[w2pool.tile([128, NTB, 128], BF, tag=f"w2b_{ig}_{og}",
                             name=f"w2b_{ig}_{og}")
                 for og in range(2)] for ig in range(2)]
        w2ft = [w2pool.tile([128, TF8, 2, 128], mybir.dt.float8e4,
                            tag=f"w2f_{og}", name=f"w2f_{og}")
                for og in range(2)]

        # ================= DMA issue block =================
        # Two HWDGE queues (sync + scalar); scalar's queue stays SHORT (5
        # early issues, no WAR waits) so its relu/exp compute never queues
        # behind DMA issues.  w1 + pixel-sliced im0 pieces lead on both
        # queues so conv1(0) can start ~11us in; w2 og0 pieces follow in
        # tap-consumption order (slice-precise dep tracking unblocks
        # conv2's taps as pieces land).  gpsimd/SWDGE takes the small
        # routing consts.
        for m in range(2):
            nc.gpsimd.dma_start(b1t[m], b1_d[m * 128:(m + 1) * 128, :])
        for m in range(2):
            nc.gpsimd.dma_start(b2t[m], b2_d[m * 128:(m + 1) * 128, :])
        nc.gpsimd.dma_start(idf, idf_d[:, :])
        nc.gpsimd.dma_start(idb, idb_d[:, :])
        for m in range(2):
            nc.gpsimd.dma_start(maskg[m], maskg_d[m])
        for m in range(2):
            nc.gpsimd.dma_start(ws_t[m], ws_d[m * 128:(m + 1) * 128, :])
        nc.gpsimd.dma_start(wcf, wcf_d[:, :, :])
        nc.gpsimd.dma_start(m80, m80_d[:, :, :])
        nc.gpsimd.dma_start(m80b, m80b_d[:, :, :])
        nc.gpsimd.dma_start(m580, m580_d[:, :])

        im = [None] * B

        def issue_im(b):
            imt = impool.tile([128, 2, NPIX1], mybir.dt.float8e4, tag="imA",
                              name="imA")
            nc.sync.dma_start(imt.rearrange("p j n -> p (j n)"),
                              x_d[b].rearrange("p j n -> p (j n)"))
            im[b] = imt

        def w2b_piece(eng, ig, og, t0, t1):
            a, bb = t0 - TF8, t1 - TF8
            eng.dma_start(
                w2bt[ig][og][:, a:bb].rearrange("p t m -> p (t m)"),
                w2b_d[ig, og, :, a * 128:bb * 128])

        PXA, PXS = 448, 1792  # im0 pixel splits (chunk n=0 | n=1-3 | n=4-6)
        im0 = impool.tile([128, 2, NPIX1], mybir.dt.float8e4, tag="imA",
                          name="imA")
        im[0] = im0
        # scalar queue: w1 + im0 front-low + og1 (consumed first) pieces;
        # front im0 pieces split per conv1 chunk so n=0 lands first, and
        # the leading w2b piece is small so conv2's tap stream starts early
        nc.scalar.dma_start(w1t.rearrange("p j n -> p (j n)"),
                            w1_d.rearrange("p j n -> p (j n)"))
        nc.scalar.dma_start(im0[0:64, :, 0:PXA], x_d[0, 0:64, :, 0:PXA])
        nc.scalar.dma_start(im0[0:64, :, PXA:PXS], x_d[0, 0:64, :, PXA:PXS])
        w2b_piece(nc.scalar, 0, 1, TF8, 26)
        w2b_piece(nc.scalar, 0, 1, 26, 46)
        w2b_piece(nc.scalar, 1, 1, 46, 64)
        w2b_piece(nc.scalar, 1, 1, 64, 81)
        w2b_piece(nc.scalar, 0, 0, TF8, 46)
        # sync queue: im0 pieces -> rest of og1 -> fp8 -> og0 -> im1-3
        nc.sync.dma_start(im0[64:128, :, 0:PXA], x_d[0, 64:128, :, 0:PXA])
        nc.sync.dma_start(im0[64:128, :, PXA:PXS], x_d[0, 64:128, :, PXA:PXS])
        w2b_piece(nc.sync, 1, 1, TF8, 26)
        nc.sync.dma_start(im0[0:64, :, PXS:NPIX1], x_d[0, 0:64, :, PXS:NPIX1])
        nc.sync.dma_start(im0[64:128, :, PXS:NPIX1],
                          x_d[0, 64:128, :, PXS:NPIX1])
        w2b_piece(nc.sync, 1, 1, 26, 46)
        w2b_piece(nc.sync, 0, 1, 46, 81)
        nc.sync.dma_start(w2ft[1].rearrange("p t j m -> p (t j m)"), w2f_d[1])
        issue_im(1)
        nc.sync.dma_start(w2ft[0].rearrange("p t j m -> p (t j m)"), w2f_d[0])
        w2b_piece(nc.sync, 1, 0, TF8, 46)
        w2b_piece(nc.sync, 0, 0, 46, 81)
        w2b_piece(nc.sync, 1, 0, 46, 81)
        issue_im(2)
        issue_im(3)

        # ================= stage pump =================
        pending = deque()
        tapctr = [0]
        STAGE_START, STAGE_EVERY = 16, 52

        def pump():
            tapctr[0] += 1
            if (pending and tapctr[0] >= STAGE_START
                    and (tapctr[0] - STAGE_START) % STAGE_EVERY == 0
                    and tapctr[0] >= pending[0][1]):
                pending.popleft()[0]()

        def drain():
            while pending:
                pending.popleft()[0]()

        # ================= per-image phases =================
        def conv1(b):
            # fp8 DoubleRow: K=256 (two 128-row k-groups) per instruction;
            # host pre-scales x by 16 and w1 by 256, undone by the relu
            # drain's 2^-12 activation scale.  n-outer so pixel chunks are
            # consumed in DMA arrival order.
            imt = im[b]
            h1t = h1pool.tile([128, 2, 56, 2, 28], BF, tag="h1t", name="h1t")
            for n in range(7):
                for m in range(2):
                    ps = cpsum.tile([128, 8, 56], F32, tag="cps", name="c1ps")
                    nc.tensor.matmul(ps, w1t[:, :, m * 128:(m + 1) * 128],
                                     imt[:, :, n * N1CH:(n + 1) * N1CH],
                                     start=True, stop=True,
                                     perf_mode=mybir.MatmulPerfMode.DoubleRow)
                    # single drain per (n, m): phase-interleave via strided
                    # view; alternate ACT/DVE so drains never pace conv1.
                    ps_v = ps.rearrange("p r (x2 ph) -> p r ph x2", ph=2)
                    dst = h1t[:, m, 8 * n:8 * n + 8, :, :]
                    if n % 2 == 0:
                        nc.scalar.activation(dst, ps_v, AF.Relu,
                                             bias=b1t[m], scale=2.0 ** -12)
                    else:
                        tmp = dpool.tile([128, 8, 2, 28], F32, tag="c1t",
                                         name="c1t")
                        nc.vector.tensor_scalar(
                            tmp, ps_v, 2.0 ** -12, b1t[m],
                            op0=mybir.AluOpType.mult,
                            op1=mybir.AluOpType.add)
                        nc.vector.tensor_scalar_max(dst, tmp, 0.0)
            return h1t

        def h1cast(h1t):
            # fp8 copy of h1 (x16) for the DoubleRow taps; one DVE op
            # (~3.6us), hidden under conv2's leading bf16 taps.  gpsimd
            # takes ~14ns/elem for fp8 stores -- keep it away from this.
            hf = h8pool.tile([128, 2, 56, 2, 28], mybir.dt.float8e4,
                             tag="h1f8", name="h1f8")
            nc.vector.tensor_scalar_mul(hf, h1t, 16.0)
            return hf

        def conv2(b, h1t, h1f8, craw):
            # 4 single-bank phases (og, y), og1 FIRST so og1's full squash
            # half can pump into og0's tap windows.  bf16 taps first (w2
            # x4096), fp8 DoubleRow taps last (so the h1 cast hides under
            # bf16); both accumulate at the same 2^12 scale, undone in the
            # drain.
            tapctr[0] = 0
            for og in (1, 0):
                for y in range(2):
                    ps = cpsum.tile([128, 288], F32, tag="cps", name="c2ps")
                    for t81 in range(TF8, 81):
                        kh, kw = t81 // 9, t81 % 9
                        for ig in range(2):
                            rhs = h1t[:, ig,
                                      kh + 24 * y:kh + 24 * y + 24:2,
                                      kw % 2, kw // 2:kw // 2 + 24]
                            nc.tensor.matmul(
                                ps, w2bt[ig][og][:, t81 - TF8, :], rhs,
                                start=(t81 == TF8 and ig == 0), stop=False)
                            pump()
                    for t81 in range(TF8):
                        kh, kw = t81 // 9, t81 % 9
                        rhs = h1f8[:, :,
                                   kh + 24 * y:kh + 24 * y + 24:2,
                                   kw % 2, kw // 2:kw // 2 + 24]
                        nc.tensor.matmul(
                            ps, w2ft[og][:, t81], rhs,
                            start=False, stop=(t81 == TF8 - 1),
                            perf_mode=mybir.MatmulPerfMode.DoubleRow)
                        pump()
                    # drain on DVE (keeps scalar's act table on Exp)
                    nc.vector.tensor_scalar(
                        craw[:, og, y * 288:(y + 1) * 288], ps,
                        2.0 ** -12, b2t[og],
                        op0=mybir.AluOpType.mult, op1=mybir.AluOpType.add)

        sqst = {}

        def scale_half(pms, nsqs, capsbf, r0, r1):
            # scale = n * rsqrt(n) / (1+n) for routes r0:r1, all on DVE,
            # chunk-packed; then scale the caps into capsbf (DVE/gpsimd).
            rn = r1 - r0
            nh = nsqs[:, :, r0:r1]
            sqas = dpool.tile([128, 5, 16], F32, tag="sqas", name="sqas")
            rys = dpool.tile([128, 5, 16], F32, tag="rys", name="rys")
            nt = dpool.tile([128, 5, 16], F32, tag="nt", name="nt")
            nc.vector.tensor_scalar(
                rys.bitcast(mybir.dt.uint32), nh.bitcast(mybir.dt.uint32),
                1, None, op0=mybir.AluOpType.logical_shift_right)
            nc.vector.tensor_tensor(
                rys.bitcast(mybir.dt.uint32),
                magic5[:, :, r0:r1].bitcast(mybir.dt.uint32),
                rys.bitcast(mybir.dt.uint32), op=mybir.AluOpType.subtract)
            nc.vector.tensor_mul(nt, rys, rys)
            nc.vector.tensor_mul(nt, nt, nh)
            nc.vector.tensor_scalar(
                nt, nt, -0.5, 1.5,
                op0=mybir.AluOpType.mult, op1=mybir.AluOpType.add)
            nc.vector.tensor_mul(rys, rys, nt)
            nc.vector.tensor_scalar_add(sqas, nh, 1.0)
            nc.vector.reciprocal(sqas, sqas)
            nc.vector.tensor_mul(sqas, sqas, rys)
            nc.vector.tensor_mul(sqas, nh, sqas)
            for k, (p0, ln) in enumerate(PIX_CHUNKS):
                pm3 = pms[k].rearrange("p (r i) -> p r i", i=D)
                cbf3 = capsbf[:, k].rearrange("p (r i) -> p r i", i=D)
                eng = nc.gpsimd if k < 2 else nc.vector
                eng.tensor_mul(
                    cbf3[:ln, r0:r1], pm3[:ln, r0:r1],
                    sqas[:ln, k].unsqueeze(2).broadcast_to([ln, rn, D]))

        def squash_A(b, craw):
            # pumped og1 squash part 1: fwd transposes + |.|^2 partials.
            # PE ops lead, DVE follows -> no PE stall at the pump slot.
            def f():
                pms = [pmpool.tile([128, 256], F32, tag="pm", name="pm")
                       for _ in PIX_CHUNKS]
                nsqs = dpool.tile([128, 5, R], F32, tag="nsqs", name="nsqs")
                for k, (p0, ln) in enumerate(PIX_CHUNKS):
                    tp = tps.tile([128, 128], BF, tag="tp", name="tp")
                    nc.tensor.transpose(tp[:ln, :], craw[:, 1, p0:p0 + ln],
                                        idb)
                    nc.scalar.activation(pms[k][:ln, 128:256], tp[:ln, :],
                                         AF.Copy)
                for k, (p0, ln) in enumerate(PIX_CHUNKS):
                    pm3 = pms[k].rearrange("p (r i) -> p r i", i=D)
                    sq = dpool.tile([128, 16, D], F32, tag="sqh", name="sqh")
                    eng = nc.gpsimd if k < 2 else nc.vector
                    eng.tensor_mul(sq[:ln], pm3[:ln, 16:32], pm3[:ln, 16:32])
                    nc.vector.reduce_sum(nsqs[:ln, k, 16:32], sq[:ln],
                                         axis=AX.X)
                sqst[b] = [pms, nsqs]
            return f

        def squash_B(b):
            # pumped og1 squash part 2: scale chain + caps scaling (no PE).
            def f():
                pms, nsqs = sqst[b]
                capsbf = capspool.tile([128, 5, 256], BF, tag="cbf",
                                       name="cbf")
                scale_half(pms, nsqs, capsbf, 16, 32)
                sqst[b] = [pms, nsqs, capsbf]
            return f

        def squash_C(b):
            # pumped og1 squash part 3: bwd transposes (deps long done ->
            # PE never waits) + capsT og1 + capsum g1.
            def f():
                pms, nsqs, capsbf = sqst[b]
                capsT = capspool.tile([128, 2, PIX], BF, tag="cT", name="cT")
                capsum = capspool.tile([128, 2], F32, tag="csum",
                                      name="csum")
                for k, (p0, ln) in enumerate(PIX_CHUNKS):
                    tb = tps.tile([128, 128], BF, tag="tp", name="tb")
                    nc.tensor.transpose(
                        tb[:, :ln], capsbf[:ln, k, 128:256], idb[:ln, :ln])
                    nc.vector.tensor_copy(capsT[:, 1, p0:p0 + ln],
                                          tb[:, :ln])
                nc.vector.reduce_sum(capsum[:, 1:2], capsT[:, 1], axis=AX.X)
                sqst[b] = [pms, nsqs, capsbf, capsT, capsum]
            return f

        def squash_D(b, craw):
            # pumped og0 squash head for pixel chunks 0-1 (their craw half
            # is og0-y0, drained one phase before conv2 ends).
            def f():
                pms, nsqs = sqst[b][0], sqst[b][1]
                for k in (0, 1):
                    p0, ln = PIX_CHUNKS[k]
                    tp = tps.tile([128, 128], BF, tag="tp", name="tp")
                    nc.tensor.transpose(tp[:ln, :], craw[:, 0, p0:p0 + ln],
                                        idb)
                    nc.vector.tensor_copy(pms[k][:ln, 0:128], tp[:ln, :])
                for k in (0, 1):
                    p0, ln = PIX_CHUNKS[k]
                    pm3 = pms[k].rearrange("p (r i) -> p r i", i=D)
                    sq = dpool.tile([128, 16, D], F32, tag="sqh", name="sqh")
                    nc.gpsimd.tensor_mul(sq[:ln], pm3[:ln, 0:16],
                                         pm3[:ln, 0:16])
                    nc.vector.reduce_sum(nsqs[:ln, k, 0:16], sq[:ln],
                                         axis=AX.X)
            return f

        def squash_finish(b, craw):
            # og0 tail remainder: chunks 2-4 + scale + bwd transposes +
            # capsum g0.
            pms, nsqs, capsbf, capsT, capsum = sqst.pop(b)
            for k in (2, 3, 4):
                p0, ln = PIX_CHUNKS[k]
                tp = tps.tile([128, 128], BF, tag="tp", name="tp")
                nc.tensor.transpose(tp[:ln, :], craw[:, 0, p0:p0 + ln], idb)
                nc.vector.tensor_copy(pms[k][:ln, 0:128], tp[:ln, :])
            for k in (2, 3, 4):
                p0, ln = PIX_CHUNKS[k]
                pm3 = pms[k].rearrange("p (r i) -> p r i", i=D)
                sq = dpool.tile([128, 16, D], F32, tag="sqh", name="sqh")
                eng = nc.gpsimd if k < 3 else nc.vector
                eng.tensor_mul(sq[:ln], pm3[:ln, 0:16], pm3[:ln, 0:16])
                nc.vector.reduce_sum(nsqs[:ln, k, 0:16], sq[:ln],
                                     axis=AX.X)
            scale_half(pms, nsqs, capsbf, 0, 16)
            for k, (p0, ln) in enumerate(PIX_CHUNKS):
                tb = tps.tile([128, 128], BF, tag="tp", name="tb")
                nc.tensor.transpose(
                    tb[:, :ln], capsbf[:ln, k, 0:128], idb[:ln, :ln])
                nc.scalar.activation(capsT[:, 0, p0:p0 + ln], tb[:, :ln],
                                     AF.Copy)
            nc.vector.reduce_sum(capsum[:, 0:1], capsT[:, 0], axis=AX.X)
            return capsbf, capsT, capsum

        # ================= routing (per image, staged) =================
        def register_routing(b, capsbf, capsT, capsum):
            st = {}
            blog = rpool.tile([128, 5, R, C], F32, tag="blog", name="blog")

            def sv_t4(it, last=False):
                # (c,o)-packed layout: partition p = c_local*16+o, halves
                # h=0 (c 0-4) / h=1 (c 5-9).  The per-class s/T4 matmuls
                # batch into 4 + 4 wide ones; the c==c' diagonal is pulled
                # out with a mask multiply + reduce on DVE.
                def f():
                    sF = dpool.tile([80, 2], F32, tag="sF", name="sF")
                    if it == 0:
                        csb = dpool.tile([128, 2], BF, tag="csb", name="csb")
                        nc.vector.tensor_scalar_mul(csb, capsum, 1.0 / C)
                        sps = rps.tile([80, 2], F32, tag="rps", name="sps0")
                        for h in range(2):
                            for m in range(2):
                                nc.tensor.matmul(
                                    sps[:, h:h + 1],
                                    ws_t[m][:, 80 * h:80 * (h + 1)],
                                    csb[:, m:m + 1],
                                    start=(m == 0), stop=(m == 1))
                        nc.vector.tensor_copy(sF, sps)
                    else:
                        Gp = st['Gp']
                        for h in range(2):
                            sps = rps.tile([80, C], F32, tag="rps",
                                           name=f"sps{h}")
                            for m in range(2):
                                nc.tensor.matmul(
                                    sps, ws_t[m][:, 80 * h:80 * (h + 1)],
                                    Gp[m], start=(m == 0), stop=(m == 1))
                            fm = dpool.tile([80, C], F32, tag="sfm",
                                            name="sfm")
                            nc.vector.tensor_mul(fm, sps, m80[:, h])
                            nc.vector.reduce_sum(sF[:, h:h + 1], fm,
                                                 axis=AX.X)
                    # squash on the packed layout: per-class norms via a
                    # block-ones matmul, scale chain on [5, 2], broadcast
                    # back via a K=5 matmul.  All elementwise on DVE.
                    sq2 = dpool.tile([80, 2], F32, tag="sq2", name="sq2")
                    nc.vector.tensor_mul(sq2, sF, sF)
                    n2ps = rps.tile([5, 2], F32, tag="rps", name="n2ps")
                    for h in range(2):
                        nc.tensor.matmul(n2ps[:, h:h + 1], m80[:, 0, :5],
                                         sq2[:, h:h + 1],
                                         start=True, stop=True)
                    n2 = dpool.tile([5, 2], F32, tag="n2", name="n2")
                    nc.vector.tensor_copy(n2, n2ps)
                    ry = dpool.tile([128, 16], F32, tag="ry", name="ry")
                    dve_rsqrt(ry[:5, :2], n2, 5, 2, "vr", iters=1)
                    a2 = dpool.tile([5, 2], F32, tag="a2", name="a2")
                    nc.vector.tensor_scalar_add(a2, n2, 1.0)
                    nc.vector.reciprocal(a2, a2)
                    nc.vector.tensor_mul(a2, a2, ry[:5, :2])
                    nc.vector.tensor_mul(a2, n2, a2)
                    scps = rps.tile([80, 2], F32, tag="rps", name="scps")
                    for h in range(2):
                        nc.tensor.matmul(scps[:, h:h + 1], m580,
                                         a2[:, h:h + 1],
                                         start=True, stop=True)
                    if last:
                        vff = dpool.tile([80, 2], F32, tag="vff", name="vff")
                        nc.vector.tensor_mul(vff, sF, scps)
                        vT = rps.tile([2, 80], F32, tag="rps", name="vT")
                        nc.tensor.transpose(vT, vff, idf[:80, :80])
                        vout = rpool.tile([2, 80], F32, tag="vout",
                                          name="vout")
                        nc.vector.tensor_copy(vout, vT)
                        nc.sync.dma_start(
                            vout_d[b * C:(b + 1) * C, :]
                            .rearrange("(h f) o -> h (f o)", f=5), vout)
                        return
                    vF2 = rpool.tile([80, 2], BF, tag="vF2", name="vF2")
                    nc.vector.tensor_mul(vF2, sF, scps)
                    # T4: rhs = vF broadcast masked to [80, C] per half,
                    # contraction over the packed (c,o) dim in 2 halves.
                    T4 = [rpool.tile([128, R * C], BF, tag=f"T4_{m}",
                                     name=f"T4_{m}") for m in range(2)]
                    vm = dpool.tile([80, 2, C], BF, tag="vm", name="vm")
                    nc.vector.tensor_mul(
                        vm, vF2.unsqueeze(2).broadcast_to([80, 2, C]), m80b)
                    for m in range(2):
                        t4 = rps.tile([128, C], F32, tag="rps", name="t4")
                        for h in range(2):
                            nc.tensor.matmul(
                                t4, wcf[:, h, m * 128:(m + 1) * 128],
                                vm[:, h], start=(h == 0), stop=(h == 1))
                        data = t4.unsqueeze(1).broadcast_to([128, R, C])
                        mk = maskg[m].rearrange("p (r c) -> p r c", c=C)
                        nc.vector.tensor_mul(
                            T4[m].rearrange("p (r c) -> p r c", c=C),
                            data, mk)
                    st['T4'] = T4
                return f

            def dlstage(it):
                def f():
                    T4 = st['T4']
                    for k, (p0, ln) in enumerate(PIX_CHUNKS):
                        dl = rps.tile([128, R, C], F32, tag="rps", name="dl")
                        for kc in range(2):
                            nc.tensor.matmul(
                                dl[:ln], capsT[:, kc, p0:p0 + ln],
                                T4[kc], start=(kc == 0), stop=(kc == 1))
                        if it == 0:
                            nc.vector.tensor_copy(blog[:ln, k], dl[:ln])
                        else:
                            nc.vector.tensor_add(blog[:ln, k], blog[:ln, k],
                                                 dl[:ln])
                return f

            def efstage():
                # exp/softmax-weight + F' matmuls, chunk-pipelined so the
                # F4 accumulation follows each chunk's cw by ~1 op instead
                # of waiting for all five chunks.
                e = rpool.tile([128, 5, R, C], BF, tag="e", name="e")
                cwt = rpool.tile([128, 5, R, D], BF, tag="cw", name="cw")
                F4 = [rps.tile([128, R * C], F32, tag="rps", name=f"F4_{m}")
                      for m in range(2)]
                dens = dpool.tile([128, 5, R], F32, tag="dens", name="dens")
                for k, (p0, ln) in enumerate(PIX_CHUNKS):
                    nc.scalar.activation(e[:ln, k], blog[:ln, k], AF.Exp)
                    nc.vector.reduce_sum(dens[:ln, k], e[:ln, k], axis=AX.X)
                    nc.vector.reciprocal(dens[:ln, k], dens[:ln, k])
                for k, (p0, ln) in enumerate(PIX_CHUNKS):
                    cbf4 = capsbf[:, k].rearrange("p (r i) -> p r i", i=D)
                    eng = nc.gpsimd if k < 2 else nc.vector
                    eng.tensor_mul(
                        cwt[:ln, k], cbf4[:ln],
                        dens[:ln, k].unsqueeze(2).broadcast_to([ln, R, D]))
                    cwf = cwt[:, k].rearrange("p r i -> p (r i)")
                    ef = e[:, k].rearrange("p r c -> p (r c)")
                    for m in range(2):
                        nc.tensor.matmul(F4[m],
                                         cwf[:ln, m * 128:(m + 1) * 128],
                                         ef[:ln],
                                         start=(k == 0), stop=(k == 4))
                Gp = [rpool.tile([128, C], BF, tag=f"G{m}", name=f"G{m}")
                      for m in range(2)]
                for m in range(2):
                    fm = dpool.tile([128, R * C], BF, tag="fm", name="fm")
                    nc.vector.tensor_mul(fm, F4[m], maskg[m])
                    gf = dpool.tile([128, C], F32, tag="gf", name="gf")
                    nc.vector.reduce_sum(
                        gf, fm.rearrange("p (r c) -> p c r", c=C), axis=AX.X)
                    nc.vector.tensor_copy(Gp[m], gf)
                st['Gp'] = Gp

            pending.extend([
                (sv_t4(0), 0), (dlstage(0), 0), (efstage, 0),
                (sv_t4(1), 0), (dlstage(1), 0), (efstage, 0),
                (sv_t4(2, last=True), 0),
            ])

        # ================= main pipeline =================
        h1s = {0: conv1(0)}
        for b in range(B):
            craw = crawpool.tile([128, 2, PIX], BF, tag="craw", name="craw")
            hf = h1cast(h1s[b])
            pending.append((squash_A(b, craw), 300))
            pending.append((squash_B(b), 0))
            pending.append((squash_C(b), 0))
            pending.append((squash_D(b, craw), 444))
            conv2(b, h1s[b], hf, craw)
            if b + 1 < B:
                h1s[b + 1] = conv1(b + 1)
            drain()  # safety: all pumped stages must be fully emitted
            cbs = squash_finish(b, craw)
            register_routing(b, *cbs)
        drain()  # routing of the last image (exposed tail)

    nc.compile()
    return nc


@functools.lru_cache(maxsize=1)
def _get_nc():
    return _build_nc()


def _prep_consts(conv1_w, conv1_b, conv2_w, conv2_b, route_w):
    bf = ml_dtypes.bfloat16
    f8 = ml_dtypes.float8_e4m3
    f32 = np.float32
    w1 = np.zeros((256, 256), f32)
    w1[:K1] = conv1_w.astype(f32).transpose(1, 2, 3, 0).reshape(K1, 256)
    # DoubleRow lhsT layout [p, j, m] = w1[j*128+p, m], fp8 with x256 scale
    w1dr = np.clip(w1 * 256.0, -240, 240).reshape(2, 128, 256).transpose(1, 0, 2)
    w2 = conv2_w.astype(f32).reshape(2, 128, 2, 128, 81)  # [og, mo, ig, ki, tap]
    # bf16 taps TF8..80: [ig, og, ki, tap, mo], x4096 (exact pow2) so they
    # accumulate at the same scale as the fp8 (x16 * x256) taps
    w2b = (w2[..., TF8:].transpose(2, 0, 3, 4, 1) * 4096.0)
    # fp8 taps 0..TF8-1: [og, ki, tap, ig, mo], x256
    w2f = np.clip(w2[..., :TF8].transpose(0, 3, 4, 2, 1) * 256.0, -240, 240)
    ws = route_w.astype(f32).transpose(0, 2, 1, 3).reshape(256, C * O)
    # wcf[(c_l,o), h, q] = wcob[o, 5h+c_l, q]; wcob[o,c,q] = route_w view
    wcob = route_w.astype(f32).transpose(3, 1, 0, 2).reshape(O, C, 256)
    wcf = np.zeros((80, 2, 256), f32)
    for cl in range(5):
        for o in range(O):
            for h in range(2):
                wcf[cl * 16 + o, h] = wcob[o, 5 * h + cl]
    # m80[p=(c_l,o), h, c'] = (c' == 5h + c_l)
    m80 = np.zeros((80, 2, C), f32)
    for cl in range(5):
        for h in range(2):
            m80[cl * 16:(cl + 1) * 16, h, 5 * h + cl] = 1.0
    # m580[j, p] = (j == p//16)
    m580 = np.zeros((5, 80), f32)
    for j in range(5):
        m580[j, j * 16:(j + 1) * 16] = 1.0
    maskg = np.zeros((2, 128, R * C), f32)
    for m in range(2):
        for j in range(128):
            r = m * 16 + j // D
            maskg[m, j, r * C:(r + 1) * C] = 1.0
    return {
        "w1": np.ascontiguousarray(w1dr).astype(f8),
        "b1": np.ascontiguousarray(conv1_b.astype(f32).reshape(256, 1)),
        "w2b": np.ascontiguousarray(w2b).reshape(2, 2, 128, NTB * 128).astype(bf),
        "w2f": np.ascontiguousarray(w2f).reshape(2, 128, TF8 * 2 * 128).astype(f8),
        "b2": np.ascontiguousarray(conv2_b.astype(f32).reshape(256, 1)),
        "ws": np.ascontiguousarray(ws).astype(bf),
        "wcf": np.ascontiguousarray(wcf).astype(bf),
        "m80": m80,
        "m80b": np.ascontiguousarray(m80).astype(bf),
        "m580": m580,
        "idf": np.eye(128, dtype=f32),
        "idb": np.eye(128, dtype=f32).astype(bf),
        "maskg": maskg,
    }


def _ensure_ntff_hook():
    """The agent image's antenv lacks axon_hooks; shim it so trace=True works."""
    import sys
    import types
    try:
        from antenv import axon_hooks  # noqa: F401
        return
    except ImportError:
        pass
    mod = types.ModuleType("antenv.axon_hooks")
    _h = [None]
    mod.get_axon_ntff_profile_hook = lambda: _h[0]
    mod.set_axon_ntff_profile_hook = lambda h: _h.__setitem__(0, h)
    sys.modules["antenv.axon_hooks"] = mod
    try:
        from trn_agent_boot.trn_boot import _ntff_profile_via_ctypes
        mod.set_axon_ntff_profile_hook(
            _ntff_profile_via_ctypes("/opt/axon/libaxon_pjrt.so"))
    except Exception as e:  # degrade: trace skipped, run still works
        print(f"ntff hook shim failed: {e}")


def run(x, conv1_w, conv1_b, conv2_w, conv2_b, route_w, trace=False, cores=NCORES):
    if trace:
        _ensure_ntff_hook()
    x = np.asarray(x, np.float32)
    nb = x.shape[0]
    consts = _prep_consts(np.asarray(conv1_w), np.asarray(conv1_b),
                          np.asarray(conv2_w), np.asarray(conv2_b),
                          np.asarray(route_w))
    win = np.lib.stride_tricks.sliding_window_view(x, (9, 9), axis=(2, 3))
    xb = (win.transpose(0, 1, 4, 5, 2, 3)          # [b, c, kh, kw, y, x]
          .reshape(nb, K1, NPIX1))
    # fp8 DoubleRow layout [b, p, j, n], rows 243..255 zero, x16 scale
    xq = np.zeros((nb, 256, NPIX1), np.float32)
    xq[:, :K1] = np.clip(xb * 16.0, -240, 240)
    xq = np.ascontiguousarray(
        xq.reshape(nb, 2, 128, NPIX1).transpose(0, 2, 1, 3)
    ).astype(ml_dtypes.float8_e4m3)
    assert nb == B * cores
    in_maps = []
    for cid in range(cores):
        m = dict(consts)
        m["x"] = np.ascontiguousarray(xq[cid * B:(cid + 1) * B])
        in_maps.append(m)
    res = run_bass_kernel_spmd(_get_nc(), in_maps, list(range(cores)), trace=trace)
    out = np.concatenate([r["v_out"].reshape(B, C, O) for r in res.results], axis=0)
    return out.astype(np.float32), res


def kernel(x, conv1_w, conv1_b, conv2_w, conv2_b, route_w):
    out, _ = run(x, conv1_w, conv1_b, conv2_w, conv2_b, route_w, trace=False)
    return out
